# revision 10
# baseline (speedup 1.0000x reference)
"""DimeNet-style GNN message passing on 8 Trainium2 NeuronCores.

Strategy
--------
Only rows dst<N of the [E,H] triplet-aggregation buffer are ever read
(agg_e[dst] with dst in [0,N)), so triplets with j_idx >= N are dead:
~40k of 640k triplets survive.

Sharding: core c owns node range [2048c, 2048(c+1)).  Triplets are
bucketed by j_idx//128 (node chunk), edges by dst//128.  All segment
sums become chunk-local one-hot matmuls accumulated in PSUM.  The only
collectives are an AllGather of the updated node table after layers 0/1
and a tiny AllReduce of pooled per-graph sums.

Gathers h[k_idx] / h[src] use dma_gather (transpose mode) from a bf16
[16384, 128] zero-padded node table in HBM, producing feature-major
tiles that feed TensorE directly.
"""
import sys

if '/opt/trn_rl_repo' not in sys.path:
    sys.path.insert(0, '/opt/trn_rl_repo')

import numpy as np
import ml_dtypes

import concourse.bacc as bacc
import concourse.mybir as mybir
import concourse.tile as tile
from concourse.bass_utils import run_bass_kernel_spmd

BF16 = ml_dtypes.bfloat16
F32 = np.float32

N, E, T, B = 16000, 256000, 640000, 128
FIN, H, OUT, L = 64, 64, 32, 3
NCORES = 8
NCH = 16                 # node chunks per core (128 nodes each)
CN = NCH * 128           # 2048 nodes per core
NPAD = NCORES * CN       # 16384 padded node table rows
NCHG = NCORES * NCH      # 128 global chunks

AF = mybir.ActivationFunctionType
ALU = mybir.AluOpType
DT = mybir.dt


def _wrap_idx(ids: np.ndarray) -> np.ndarray:
    """dma_gather index layout: idx i -> [i%16, i//16], replicated to 128 partitions."""
    s = ids.shape[0]
    w = ids.reshape(s // 16, 16).T.astype(np.int16)
    return np.ascontiguousarray(np.tile(w, (8, 1)))


def _bucket(gchunk: np.ndarray, cap: int, nvals: int):
    """Slot position for each element: gchunk*cap + rank-within-chunk."""
    order = np.argsort(gchunk, kind='stable')
    sg = gchunk[order]
    starts = np.searchsorted(sg, np.arange(NCHG))
    rank = np.arange(len(sg)) - starts[sg]
    assert rank.max() < cap if len(rank) else True
    pos = sg * cap + rank
    out_pos = np.empty(nvals, np.int64)
    out_pos[order] = pos
    return out_pos


def _build_program(TA: int, TB: int):
    CAPA, CAPB = TA * 128, TB * 128
    nc = bacc.Bacc("TRN2", target_bir_lowering=False, debug=False,
                   num_devices=NCORES)

    # ---------------- DRAM I/O ----------------
    d = {}
    def din(name, shape, dt):
        d[name] = nc.dram_tensor(name, list(shape), dt, kind="ExternalInput")
        return d[name]

    xpad = din("xpad", [NPAD, 128], DT.bfloat16)
    xT = din("xT", [64, CN], DT.bfloat16)
    a_idx = din("a_idx", [128, NCH * CAPA // 16], DT.int16)
    a_cmp = din("a_cmp", [128, NCH * TA], DT.float32)
    a_rc = din("a_rc", [16, NCH * CAPA], DT.bfloat16)
    b_idx = din("b_idx", [128, NCH * CAPB // 16], DT.int16)
    b_cmp = din("b_cmp", [128, NCH * TB], DT.float32)
    b_dst = din("b_dst", [128, NCH * CAPB // 16], DT.int16)
    p_cmp = din("p_cmp", [128, NCH], DT.float32)
    sel64 = din("sel64", [128, 64], DT.bfloat16)
    iota_r = din("iota_r", [128, 128], DT.bfloat16)
    ones_b = din("ones_b", [1, 128], DT.bfloat16)
    w1h = din("w1h", [L, 128, 64], DT.bfloat16)
    w1rc = din("w1rc", [L, 16, 64], DT.bfloat16)
    w2a = din("w2a", [L, 128, 64], DT.bfloat16)
    w2b = din("w2b", [L, 64, 64], DT.bfloat16)
    b2r = din("b2r", [L, 1, 64], DT.bfloat16)
    wn1a = din("wn1a", [L, 64, 64], DT.bfloat16)
    wn1b = din("wn1b", [L, 64, 64], DT.bfloat16)
    wn2 = din("wn2", [L, 64, 64], DT.bfloat16)
    bn1c = din("bn1c", [L, 64, 1], DT.float32)
    bn2c = din("bn2c", [L, 64, 1], DT.float32)
    bn2r = din("bn2r", [L, 1, 64], DT.bfloat16)
    wo1 = din("wo1", [64, 64], DT.float32)
    bo1c = din("bo1c", [64, 1], DT.float32)
    wo2 = din("wo2", [64, 32], DT.float32)
    bo2c = din("bo2c", [32, 1], DT.float32)
    invr = din("invr", [1, 128], DT.float32)
    outT = nc.dram_tensor("outT", [32, 128], DT.float32, kind="ExternalOutput")

    with tile.TileContext(nc) as tc:
        with (
            tc.tile_pool(name="res", bufs=1) as res,           # resident SBUF
            tc.tile_pool(name="gat", bufs=2) as gat,           # gather tiles
            tc.tile_pool(name="oh", bufs=3) as oh,             # one-hot tiles
            tc.tile_pool(name="sm", bufs=3) as sm,             # small evacuations
            tc.tile_pool(name="pw", bufs=3, space="PSUM") as pw,    # work psum
            tc.tile_pool(name="pa", bufs=3, space="PSUM") as pa,    # accumulators
            tc.tile_pool(name="pp", bufs=1, space="PSUM") as pp,    # pooled accum
            tc.tile_pool(name="dram", bufs=1, space="DRAM") as dram,
        ):
            # ---------- load resident data ----------
            def load(src, shape, dt, name):
                t = res.tile(list(shape), dt, name=name)
                nc.sync.dma_start(t[:], src)
                return t

            a_idx_s = load(a_idx.ap(), [128, NCH * CAPA // 16], DT.int16, "a_idx_s")
            a_cmp_s = load(a_cmp.ap(), [128, NCH * TA], DT.float32, "a_cmp_s")
            a_rc_s = load(a_rc.ap(), [16, NCH * CAPA], DT.bfloat16, "a_rc_s")
            b_idx_s = load(b_idx.ap(), [128, NCH * CAPB // 16], DT.int16, "b_idx_s")
            b_cmp_s = load(b_cmp.ap(), [128, NCH * TB], DT.float32, "b_cmp_s")
            b_dst_s = load(b_dst.ap(), [128, NCH * CAPB // 16], DT.int16, "b_dst_s")
            p_cmp_s = load(p_cmp.ap(), [128, NCH], DT.float32, "p_cmp_s")
            sel64_s = load(sel64.ap(), [128, 64], DT.bfloat16, "sel64_s")
            iota_r_s = load(iota_r.ap(), [128, 128], DT.bfloat16, "iota_r_s")
            ones_b_s = load(ones_b.ap(), [1, 128], DT.bfloat16, "ones_b_s")
            def load_l(src, parts, width, dt, name):
                """Load [L, parts, width] DRAM tensor as [parts, L*width] SBUF."""
                t = res.tile([parts, L * width], dt, name=name)
                for l in range(L):
                    nc.sync.dma_start(t[:, l * width:(l + 1) * width],
                                      src.ap()[l])
                return t

            w1h_s = load_l(w1h, 128, 64, DT.bfloat16, "w1h_s")
            w1rc_s = load_l(w1rc, 16, 64, DT.bfloat16, "w1rc_s")
            w2a_s = load_l(w2a, 128, 64, DT.bfloat16, "w2a_s")
            w2b_s = load_l(w2b, 64, 64, DT.bfloat16, "w2b_s")
            b2r_s = load_l(b2r, 1, 64, DT.bfloat16, "b2r_s")
            wn1a_s = load_l(wn1a, 64, 64, DT.bfloat16, "wn1a_s")
            wn1b_s = load_l(wn1b, 64, 64, DT.bfloat16, "wn1b_s")
            wn2_s = load_l(wn2, 64, 64, DT.bfloat16, "wn2_s")
            bn1c_s = load_l(bn1c, 64, 1, DT.float32, "bn1c_s")
            bn2c_s = load_l(bn2c, 64, 1, DT.float32, "bn2c_s")
            bn2r_s = load_l(bn2r, 1, 64, DT.bfloat16, "bn2r_s")
            wo1_s = load(wo1.ap(), [64, 64], DT.float32, "wo1_s")
            bo1c_s = load(bo1c.ap(), [64, 1], DT.float32, "bo1c_s")
            wo2_s = load(wo2.ap(), [64, 32], DT.float32, "wo2_s")
            bo2c_s = load(bo2c.ap(), [32, 1], DT.float32, "bo2c_s")
            invr_s = load(invr.ap(), [1, 128], DT.float32, "invr_s")
            hT = load(xT.ap(), [64, CN], DT.bfloat16, "hT")

            # ---------- internal DRAM ----------
            bounce = [dram.tile([CN, 128], DT.bfloat16, name=f"bounce{l}")
                      for l in range(L - 1)]
            tables = [dram.tile([NPAD, 128], DT.bfloat16, name=f"table{l}")
                      for l in range(1, L)]
            pb_in = dram.tile([64, 128], DT.float32, name="pb_in")
            pb_out = dram.tile([64, 128], DT.float32, name="pb_out")

            pooled_ps = pp.tile([64, 128], DT.float32, name="pooled_ps")

            for l in range(L):
                table_ap = xpad.ap() if l == 0 else tables[l - 1][:, :]
                lw = slice(l * 64, (l + 1) * 64)
                for ch in range(NCH):
                    # ================= stage A: triplets -> agg =================
                    # single_packet=False is required above ~512 idxs/call
                    # (single-packet mode wedges the device).
                    agA = gat.tile([128, 1, CAPA], DT.bfloat16, name="agA")
                    nc.gpsimd.dma_gather(
                        agA[:], table_ap,
                        a_idx_s[:, ch * (CAPA // 16):(ch + 1) * (CAPA // 16)],
                        CAPA, CAPA, 128, transpose=True, single_packet=False)
                    agg_ps = pa.tile([64, 128], DT.float32, name="agg_ps", tag="acc")
                    for t in range(TA):
                        em_ps = pw.tile([128, 64], DT.float32, name="em_ps", tag="w")
                        nc.tensor.matmul(em_ps[:], agA[:, 0, t * 128:(t + 1) * 128],
                                         w1h_s[:, lw], start=True, stop=False)
                        gt = ch * TA + t
                        nc.tensor.matmul(em_ps[:],
                                         a_rc_s[:, gt * 128:(gt + 1) * 128],
                                         w1rc_s[:, lw], start=False, stop=True)
                        em_sb = sm.tile([128, 64], DT.bfloat16, name="em_sb")
                        nc.scalar.activation(em_sb[:], em_ps[:], AF.Relu)
                        sj = oh.tile([128, 128], DT.bfloat16, name="sj")
                        nc.vector.tensor_scalar(sj[:], iota_r_s[:],
                                                a_cmp_s[:, gt:gt + 1], None,
                                                ALU.is_equal)
                        nc.tensor.matmul(agg_ps[:], em_sb[:], sj[:],
                                         start=(t == 0), stop=(t == TA - 1))
                    aggT_sb = sm.tile([64, 128], DT.bfloat16, name="aggT_sb")
                    nc.scalar.activation(aggT_sb[:], agg_ps[:], AF.Copy)
                    apj_ps = pw.tile([128, 64], DT.float32, name="apj_ps", tag="w")
                    nc.tensor.matmul(apj_ps[:], ones_b_s[:], b2r_s[:, lw],
                                     start=True, stop=False)
                    nc.tensor.matmul(apj_ps[:], aggT_sb[:], w2b_s[:, lw],
                                     start=False, stop=True)
                    # agg_proj rows padded to 256B for the SBUF-source gather;
                    # pad cols must be finite (zero) since SEL kills them.
                    apj_pad = sm.tile([128, 128], DT.bfloat16, name="apj_pad")
                    nc.vector.memset(apj_pad[:, 64:128], 0.0)
                    nc.scalar.activation(apj_pad[:, 0:64], apj_ps[:], AF.Copy)

                    # ================= stage B: edges -> aggr =================
                    agB = gat.tile([128, 1, CAPB], DT.bfloat16, name="agB")
                    nc.gpsimd.dma_gather(
                        agB[:], table_ap,
                        b_idx_s[:, ch * (CAPB // 16):(ch + 1) * (CAPB // 16)],
                        CAPB, CAPB, 128, transpose=True, single_packet=False)
                    # expand agg_proj[dst] via SBUF-source gather (dst_local idxs)
                    gapj = gat.tile([128, 1, CAPB], DT.bfloat16, name="gapj")
                    nc.gpsimd.dma_gather(
                        gapj[:], apj_pad[:],
                        b_dst_s[:, ch * (CAPB // 16):(ch + 1) * (CAPB // 16)],
                        CAPB, CAPB, 128, transpose=True, single_packet=False,
                        sbuf_tokens_per_rank=128, sbuf_free_dim_per_rank=256,
                        sbuf_free_dim_pad_per_rank=0, sbuf_byte_offset=0)
                    aggr_ps = pa.tile([64, 128], DT.float32, name="aggr_ps", tag="acc")
                    for t in range(TB):
                        gt = ch * TB + t
                        de = oh.tile([128, 128], DT.bfloat16, name="de")
                        nc.vector.tensor_scalar(de[:], iota_r_s[:],
                                                b_cmp_s[:, gt:gt + 1], None,
                                                ALU.is_equal)
                        fi_ps = pw.tile([128, 64], DT.float32, name="fi_ps", tag="w")
                        nc.tensor.matmul(fi_ps[:], agB[:, 0, t * 128:(t + 1) * 128],
                                         w2a_s[:, lw], start=True, stop=False)
                        nc.tensor.matmul(fi_ps[:], gapj[:, 0, t * 128:(t + 1) * 128],
                                         sel64_s[:], start=False, stop=True)
                        fi_sb = sm.tile([128, 64], DT.bfloat16, name="fi_sb")
                        nc.scalar.activation(fi_sb[:], fi_ps[:], AF.Relu)
                        nc.tensor.matmul(aggr_ps[:], fi_sb[:], de[:],
                                         start=(t == 0), stop=(t == TB - 1))
                    aggr_sb = sm.tile([64, 128], DT.bfloat16, name="aggr_sb")
                    nc.scalar.activation(aggr_sb[:], aggr_ps[:], AF.Copy)

                    # ================= stage C: node update =================
                    chs = slice(ch * 128, (ch + 1) * 128)
                    z_ps = pw.tile([64, 128], DT.float32, name="z_ps", tag="w")
                    nc.tensor.matmul(z_ps[:], wn1a_s[:, lw], hT[:, chs],
                                     start=True, stop=False)
                    nc.tensor.matmul(z_ps[:], wn1b_s[:, lw], aggr_sb[:],
                                     start=False, stop=True)
                    z_sb = sm.tile([64, 128], DT.bfloat16, name="z_sb")
                    nc.scalar.activation(z_sb[:], z_ps[:], AF.Relu,
                                         bias=bn1c_s[:, l:l + 1])
                    hn_ps = pw.tile([128, 64], DT.float32, name="hn_ps", tag="w")
                    nc.tensor.matmul(hn_ps[:], ones_b_s[:], bn2r_s[:, lw],
                                     start=True, stop=False)
                    nc.tensor.matmul(hn_ps[:], z_sb[:], wn2_s[:, lw],
                                     start=False, stop=True)
                    if l < L - 1:
                        hn_pad = sm.tile([128, 128], DT.bfloat16, name="hn_pad")
                        nc.vector.memset(hn_pad[:, 64:128], 0.0)
                        nc.scalar.activation(hn_pad[:, 0:64], hn_ps[:], AF.Copy)
                        nc.sync.dma_start(bounce[l][chs, :], hn_pad[:])
                        hT_ps = pw.tile([64, 128], DT.float32, name="hT_ps", tag="w")
                        nc.tensor.matmul(hT_ps[:], wn2_s[:, lw], z_sb[:],
                                         start=True, stop=True)
                        nc.scalar.activation(hT[:, chs], hT_ps[:], AF.Identity,
                                             bias=bn2c_s[:, l:l + 1])
                    else:
                        hn_sb = sm.tile([128, 64], DT.bfloat16, name="hn_sb")
                        nc.scalar.activation(hn_sb[:], hn_ps[:], AF.Copy)
                        bm = oh.tile([128, 128], DT.bfloat16, name="bm")
                        nc.vector.tensor_scalar(bm[:], iota_r_s[:],
                                                p_cmp_s[:, ch:ch + 1], None,
                                                ALU.is_equal)
                        nc.tensor.matmul(pooled_ps[:], hn_sb[:], bm[:],
                                         start=(ch == 0), stop=(ch == NCH - 1))
                if l < L - 1:
                    nc.gpsimd.collective_compute(
                        "AllGather", ALU.bypass,
                        replica_groups=[list(range(NCORES))],
                        ins=[bounce[l].opt()], outs=[tables[l].opt()])

            # ================= pooling + head =================
            pooled_sb = res.tile([64, 128], DT.float32, name="pooled_sb")
            nc.scalar.activation(pooled_sb[:], pooled_ps[:], AF.Copy)
            nc.sync.dma_start(pb_in[:, :], pooled_sb[:])
            nc.gpsimd.collective_compute(
                "AllReduce", ALU.add, replica_groups=[list(range(NCORES))],
                ins=[pb_in.opt()], outs=[pb_out.opt()])
            pooled_all = res.tile([64, 128], DT.float32, name="pooled_all")
            nc.sync.dma_start(pooled_all[:], pb_out[:, :])
            ones_f = res.tile([1, 64], DT.float32, name="ones_f")
            nc.vector.memset(ones_f[:], 1.0)
            inv_ps = pw.tile([64, 128], DT.float32, name="inv_ps", tag="w")
            nc.tensor.matmul(inv_ps[:], ones_f[:], invr_s[:], start=True, stop=True)
            pm_sb = res.tile([64, 128], DT.float32, name="pm_sb")
            nc.vector.tensor_tensor(pm_sb[:], pooled_all[:], inv_ps[:], ALU.mult)
            q_sb = res.tile([64, 128], DT.float32, name="q_sb")
            nc.scalar.activation(q_sb[:], pm_sb[:], AF.Relu)
            o1_ps = pw.tile([64, 128], DT.float32, name="o1_ps", tag="w")
            nc.tensor.matmul(o1_ps[:], wo1_s[:], q_sb[:], start=True, stop=True)
            o1_sb = res.tile([64, 128], DT.float32, name="o1_sb")
            nc.scalar.activation(o1_sb[:], o1_ps[:], AF.Relu, bias=bo1c_s[:])
            o2_ps = pw.tile([32, 128], DT.float32, name="o2_ps", tag="w")
            nc.tensor.matmul(o2_ps[:], wo2_s[:], o1_sb[:], start=True, stop=True)
            o2_sb = res.tile([32, 128], DT.float32, name="o2_sb")
            nc.scalar.activation(o2_sb[:], o2_ps[:], AF.Identity, bias=bo2c_s[:])
            nc.sync.dma_start(outT.ap(), o2_sb[:])
    nc.compile()
    return nc


def _pack_inputs(inputs):
    x = np.asarray(inputs["x"], F32)
    rbf = np.asarray(inputs["rbf"], F32)
    cbf = np.asarray(inputs["cbf"], F32)
    ei = np.asarray(inputs["edge_index"]).astype(np.int64)
    src, dst = ei[0], ei[1]
    k_idx = np.asarray(inputs["k_idx"]).astype(np.int64)
    j_idx = np.asarray(inputs["j_idx"]).astype(np.int64)
    batch = np.asarray(inputs["batch"]).astype(np.int64)
    W1 = np.asarray(inputs["W1"], F32); b1 = np.asarray(inputs["b1"], F32)
    W2 = np.asarray(inputs["W2"], F32); b2 = np.asarray(inputs["b2"], F32)
    Wn1 = np.asarray(inputs["Wn1"], F32); bn1 = np.asarray(inputs["bn1"], F32)
    Wn2 = np.asarray(inputs["Wn2"], F32); bn2 = np.asarray(inputs["bn2"], F32)
    Wo1 = np.asarray(inputs["Wo1"], F32); bo1 = np.asarray(inputs["bo1"], F32)
    Wo2 = np.asarray(inputs["Wo2"], F32); bo2 = np.asarray(inputs["bo2"], F32)

    # ---- triplet filtering + bucketing by j chunk ----
    keep = j_idx < N
    kk = k_idx[keep]; jk = j_idx[keep]
    rck = np.concatenate([rbf[jk], cbf[keep]], axis=1)   # [Tk, 12]

    gA = jk // 128
    cntA = np.bincount(gA, minlength=NCHG)
    TA = max(2, int(np.ceil(cntA.max() / 128)))
    CAPA = TA * 128
    posA = _bucket(gA, CAPA, len(jk))
    ak = np.zeros(NCHG * CAPA, np.int64); ak[posA] = kk
    acmp = np.full(NCHG * CAPA, -1.0, F32); acmp[posA] = (jk % 128).astype(F32)
    arc = np.zeros((NCHG * CAPA, 12), F32); arc[posA] = rck

    # ---- edge bucketing by dst chunk ----
    gB = dst // 128
    cntB = np.bincount(gB, minlength=NCHG)
    TB = max(2, int(np.ceil(cntB.max() / 128)))
    CAPB = TB * 128
    posB = _bucket(gB, CAPB, E)
    bsrc = np.zeros(NCHG * CAPB, np.int64); bsrc[posB] = src
    bcmp = np.full(NCHG * CAPB, -1.0, F32); bcmp[posB] = (dst % 128).astype(F32)
    bdst = np.zeros(NCHG * CAPB, np.int64); bdst[posB] = dst % 128

    # ---- shared (replicated) tensors ----
    xpad = np.zeros((NPAD, 128), BF16)
    xpad[:N, :64] = x.astype(BF16)
    cnt = np.bincount(batch, minlength=128).astype(F32)[:128]
    invr = (1.0 / np.maximum(cnt, 1.0))[None, :].astype(F32)
    iota_r = np.ascontiguousarray(
        np.broadcast_to(np.arange(128, dtype=F32), (128, 128))).astype(BF16)
    ones_b = np.ones((1, 128), BF16)
    sel64v = np.zeros((128, 64), BF16); sel64v[:64] = np.eye(64, dtype=BF16)

    w1h = np.zeros((L, 128, 64), BF16); w1h[:, :64] = W1[:, :64].astype(BF16)
    w1rc = np.zeros((L, 16, 64), BF16)
    w1rc[:, :12] = W1[:, 64:76].astype(BF16)
    w1rc[:, 12] = b1.astype(BF16)
    w2a = np.zeros((L, 128, 64), BF16); w2a[:, :64] = W2[:, :64].astype(BF16)
    w2b = W2[:, 64:].astype(BF16)
    b2r = b2[:, None, :].astype(BF16)
    wn1a = Wn1[:, :64].astype(BF16)
    wn1b = Wn1[:, 64:].astype(BF16)
    wn2k = Wn2.astype(BF16)
    bn1ck = bn1[:, :, None].astype(F32)
    bn2ck = bn2[:, :, None].astype(F32)
    bn2rk = bn2[:, None, :].astype(BF16)

    batch_pad = np.full(NPAD, -1.0, F32)
    batch_pad[:N] = batch.astype(F32)

    shared = dict(
        xpad=xpad, iota_r=iota_r, ones_b=ones_b, sel64=sel64v,
        w1h=w1h, w1rc=w1rc, w2a=w2a, w2b=w2b, b2r=b2r,
        wn1a=wn1a, wn1b=wn1b, wn2=wn2k, bn1c=bn1ck, bn2c=bn2ck, bn2r=bn2rk,
        wo1=Wo1, bo1c=bo1[:, None].astype(F32), wo2=Wo2,
        bo2c=bo2[:, None].astype(F32), invr=invr,
    )

    in_maps = []
    for c in range(NCORES):
        ts = slice(c * NCH * CAPA, (c + 1) * NCH * CAPA)   # triplet slots
        es = slice(c * NCH * CAPB, (c + 1) * NCH * CAPB)   # edge slots
        ns = slice(c * CN, (c + 1) * CN)                   # node slots
        a_rc_c = np.zeros((16, NCH * CAPA), BF16)
        a_rc_c[:12] = arc[ts].T.astype(BF16)
        a_rc_c[12] = 1.0
        xT_c = np.zeros((64, CN), BF16)
        nhi = min((c + 1) * CN, N)
        if nhi > c * CN:
            xT_c[:, :nhi - c * CN] = x[c * CN:nhi].T.astype(BF16)
        m = dict(shared)
        m.update(
            xT=xT_c,
            a_idx=_wrap_idx(ak[ts]),
            a_cmp=np.ascontiguousarray(
                acmp[ts].reshape(NCH * TA, 128).T),
            a_rc=a_rc_c,
            b_idx=_wrap_idx(bsrc[es]),
            b_cmp=np.ascontiguousarray(
                bcmp[es].reshape(NCH * TB, 128).T),
            b_dst=_wrap_idx(bdst[es]),
            p_cmp=np.ascontiguousarray(
                batch_pad[ns].reshape(NCH, 128).T),
        )
        in_maps.append(m)
    return TA, TB, in_maps


_PROG_CACHE = {}


def kernel(**inputs) -> np.ndarray:
    TA, TB, in_maps = _pack_inputs(inputs)
    key = (TA, TB)
    if key not in _PROG_CACHE:
        _PROG_CACHE[key] = _build_program(TA, TB)
    nc = _PROG_CACHE[key]
    res = run_bass_kernel_spmd(nc, in_maps, core_ids=list(range(NCORES)))
    return np.ascontiguousarray(res.results[0]["outT"].T).astype(F32)


# revision 11
# speedup vs baseline: 1.0593x; 1.0593x over previous
"""DimeNet-style GNN message passing on 8 Trainium2 NeuronCores.

Strategy
--------
Only rows dst<N of the [E,H] triplet-aggregation buffer are ever read
(agg_e[dst] with dst in [0,N)), so triplets with j_idx >= N are dead:
~40k of 640k triplets survive.

Sharding: core c owns node range [2048c, 2048(c+1)).  Triplets are
bucketed by j_idx//128 (node chunk), edges by dst//128.  All segment
sums become chunk-local one-hot matmuls accumulated in PSUM.  The only
collectives are an AllGather of the updated node table after layers 0/1
and a tiny AllReduce of pooled per-graph sums.

Gathers h[k_idx] / h[src] use dma_gather (transpose mode) from a bf16
[16384, 128] zero-padded node table in HBM, producing feature-major
tiles that feed TensorE directly.
"""
import sys

if '/opt/trn_rl_repo' not in sys.path:
    sys.path.insert(0, '/opt/trn_rl_repo')

import numpy as np
import ml_dtypes

import concourse.bacc as bacc
import concourse.mybir as mybir
import concourse.tile as tile
from concourse.bass_utils import run_bass_kernel_spmd

BF16 = ml_dtypes.bfloat16
F32 = np.float32

N, E, T, B = 16000, 256000, 640000, 128
FIN, H, OUT, L = 64, 64, 32, 3
NCORES = 8
NCH = 16                 # node chunks per core (128 nodes each)
CN = NCH * 128           # 2048 nodes per core
NPAD = NCORES * CN       # 16384 padded node table rows
NCHG = NCORES * NCH      # 128 global chunks

AF = mybir.ActivationFunctionType
ALU = mybir.AluOpType
DT = mybir.dt


def _wrap_idx(ids: np.ndarray) -> np.ndarray:
    """dma_gather index layout: idx i -> [i%16, i//16], replicated to 128 partitions."""
    s = ids.shape[0]
    w = ids.reshape(s // 16, 16).T.astype(np.int16)
    return np.ascontiguousarray(np.tile(w, (8, 1)))


def _bucket(gchunk: np.ndarray, cap: int, nvals: int):
    """Slot position for each element: gchunk*cap + rank-within-chunk."""
    order = np.argsort(gchunk, kind='stable')
    sg = gchunk[order]
    starts = np.searchsorted(sg, np.arange(NCHG))
    rank = np.arange(len(sg)) - starts[sg]
    assert rank.max() < cap if len(rank) else True
    pos = sg * cap + rank
    out_pos = np.empty(nvals, np.int64)
    out_pos[order] = pos
    return out_pos


def _build_program(TA: int, TB: int):
    CAPA, CAPB = TA * 128, TB * 128
    nc = bacc.Bacc("TRN2", target_bir_lowering=False, debug=False,
                   num_devices=NCORES)

    # ---------------- DRAM I/O ----------------
    d = {}
    def din(name, shape, dt):
        d[name] = nc.dram_tensor(name, list(shape), dt, kind="ExternalInput")
        return d[name]

    xpad = din("xpad", [NPAD, 128], DT.bfloat16)
    xT = din("xT", [64, CN], DT.bfloat16)
    a_idx = din("a_idx", [128, NCH * CAPA // 16], DT.int16)
    a_cmp = din("a_cmp", [128, NCH * TA], DT.float32)
    a_rc = din("a_rc", [16, NCH * CAPA], DT.bfloat16)
    b_idx = din("b_idx", [128, NCH * CAPB // 16], DT.int16)
    b_cmp = din("b_cmp", [128, NCH * TB], DT.float32)
    b_dst = din("b_dst", [128, NCH * CAPB // 16], DT.int16)
    p_cmp = din("p_cmp", [128, NCH], DT.float32)
    sel64 = din("sel64", [128, 64], DT.bfloat16)
    iota_r = din("iota_r", [128, 128], DT.bfloat16)
    ones_b = din("ones_b", [1, 128], DT.bfloat16)
    w1h = din("w1h", [L, 128, 64], DT.bfloat16)
    w1rc = din("w1rc", [L, 16, 64], DT.bfloat16)
    w2a = din("w2a", [L, 128, 64], DT.bfloat16)
    w2b = din("w2b", [L, 64, 64], DT.bfloat16)
    b2r = din("b2r", [L, 1, 64], DT.bfloat16)
    wn1a = din("wn1a", [L, 64, 64], DT.bfloat16)
    wn1b = din("wn1b", [L, 64, 64], DT.bfloat16)
    wn2 = din("wn2", [L, 64, 64], DT.bfloat16)
    bn1c = din("bn1c", [L, 64, 1], DT.float32)
    bn2c = din("bn2c", [L, 64, 1], DT.float32)
    bn2r = din("bn2r", [L, 1, 64], DT.bfloat16)
    wo1 = din("wo1", [64, 64], DT.float32)
    bo1c = din("bo1c", [64, 1], DT.float32)
    wo2 = din("wo2", [64, 32], DT.float32)
    bo2c = din("bo2c", [32, 1], DT.float32)
    invr = din("invr", [1, 128], DT.float32)
    outT = nc.dram_tensor("outT", [32, 128], DT.float32, kind="ExternalOutput")

    with tile.TileContext(nc) as tc:
        with (
            tc.tile_pool(name="res", bufs=1) as res,           # resident SBUF
            tc.tile_pool(name="gat", bufs=2) as gat,           # gather tiles
            tc.tile_pool(name="oh", bufs=3) as oh,             # one-hot tiles
            tc.tile_pool(name="sm", bufs=3) as sm,             # small evacuations
            tc.tile_pool(name="pw", bufs=3, space="PSUM") as pw,    # work psum
            tc.tile_pool(name="pa", bufs=3, space="PSUM") as pa,    # accumulators
            tc.tile_pool(name="pp", bufs=1, space="PSUM") as pp,    # pooled accum
            tc.tile_pool(name="dram", bufs=1, space="DRAM") as dram,
        ):
            # ---------- load resident data ----------
            def load(src, shape, dt, name):
                t = res.tile(list(shape), dt, name=name)
                nc.sync.dma_start(t[:], src)
                return t

            a_idx_s = load(a_idx.ap(), [128, NCH * CAPA // 16], DT.int16, "a_idx_s")
            a_cmp_s = load(a_cmp.ap(), [128, NCH * TA], DT.float32, "a_cmp_s")
            a_rc_s = load(a_rc.ap(), [16, NCH * CAPA], DT.bfloat16, "a_rc_s")
            b_idx_s = load(b_idx.ap(), [128, NCH * CAPB // 16], DT.int16, "b_idx_s")
            b_cmp_s = load(b_cmp.ap(), [128, NCH * TB], DT.float32, "b_cmp_s")
            b_dst_s = load(b_dst.ap(), [128, NCH * CAPB // 16], DT.int16, "b_dst_s")
            p_cmp_s = load(p_cmp.ap(), [128, NCH], DT.float32, "p_cmp_s")
            sel64_s = load(sel64.ap(), [128, 64], DT.bfloat16, "sel64_s")
            iota_r_s = load(iota_r.ap(), [128, 128], DT.bfloat16, "iota_r_s")
            ones_b_s = load(ones_b.ap(), [1, 128], DT.bfloat16, "ones_b_s")
            def load_l(src, parts, width, dt, name):
                """Load [L, parts, width] DRAM tensor as [parts, L*width] SBUF."""
                t = res.tile([parts, L * width], dt, name=name)
                for l in range(L):
                    nc.sync.dma_start(t[:, l * width:(l + 1) * width],
                                      src.ap()[l])
                return t

            w1h_s = load_l(w1h, 128, 64, DT.bfloat16, "w1h_s")
            w1rc_s = load_l(w1rc, 16, 64, DT.bfloat16, "w1rc_s")
            w2a_s = load_l(w2a, 128, 64, DT.bfloat16, "w2a_s")
            w2b_s = load_l(w2b, 64, 64, DT.bfloat16, "w2b_s")
            b2r_s = load_l(b2r, 1, 64, DT.bfloat16, "b2r_s")
            wn1a_s = load_l(wn1a, 64, 64, DT.bfloat16, "wn1a_s")
            wn1b_s = load_l(wn1b, 64, 64, DT.bfloat16, "wn1b_s")
            wn2_s = load_l(wn2, 64, 64, DT.bfloat16, "wn2_s")
            bn1c_s = load_l(bn1c, 64, 1, DT.float32, "bn1c_s")
            bn2c_s = load_l(bn2c, 64, 1, DT.float32, "bn2c_s")
            bn2r_s = load_l(bn2r, 1, 64, DT.bfloat16, "bn2r_s")
            wo1_s = load(wo1.ap(), [64, 64], DT.float32, "wo1_s")
            bo1c_s = load(bo1c.ap(), [64, 1], DT.float32, "bo1c_s")
            wo2_s = load(wo2.ap(), [64, 32], DT.float32, "wo2_s")
            bo2c_s = load(bo2c.ap(), [32, 1], DT.float32, "bo2c_s")
            invr_s = load(invr.ap(), [1, 128], DT.float32, "invr_s")
            hT = load(xT.ap(), [64, CN], DT.bfloat16, "hT")

            # ---------- internal DRAM ----------
            bounce = [dram.tile([CN, 128], DT.bfloat16, name=f"bounce{l}")
                      for l in range(L - 1)]
            tables = [dram.tile([NPAD, 128], DT.bfloat16, name=f"table{l}")
                      for l in range(1, L)]
            pb_in = dram.tile([64, 128], DT.float32, name="pb_in")
            pb_out = dram.tile([64, 128], DT.float32, name="pb_out")

            pooled_ps = pp.tile([64, 128], DT.float32, name="pooled_ps")

            for l in range(L):
                table_ap = xpad.ap() if l == 0 else tables[l - 1][:, :]
                lw = slice(l * 64, (l + 1) * 64)
                for ch in range(NCH):
                    # ================= stage A: triplets -> agg =================
                    # single_packet=False is required above ~512 idxs/call
                    # (single-packet mode wedges the device).
                    agA = gat.tile([128, 1, CAPA], DT.bfloat16, name="agA")
                    for s in range(0, CAPA, 512):
                        w = min(512, CAPA - s)
                        nc.gpsimd.dma_gather(
                            agA[:, :, s:s + w], table_ap,
                            a_idx_s[:, (ch * CAPA + s) // 16:
                                    (ch * CAPA + s + w) // 16],
                            w, w, 128, transpose=True)
                    agg_ps = pa.tile([64, 128], DT.float32, name="agg_ps", tag="acc")
                    for t in range(TA):
                        em_ps = pw.tile([128, 64], DT.float32, name="em_ps", tag="w")
                        nc.tensor.matmul(em_ps[:], agA[:, 0, t * 128:(t + 1) * 128],
                                         w1h_s[:, lw], start=True, stop=False)
                        gt = ch * TA + t
                        nc.tensor.matmul(em_ps[:],
                                         a_rc_s[:, gt * 128:(gt + 1) * 128],
                                         w1rc_s[:, lw], start=False, stop=True)
                        em_sb = sm.tile([128, 64], DT.bfloat16, name="em_sb")
                        nc.scalar.activation(em_sb[:], em_ps[:], AF.Relu)
                        sj = oh.tile([128, 128], DT.bfloat16, name="sj")
                        nc.vector.tensor_scalar(sj[:], iota_r_s[:],
                                                a_cmp_s[:, gt:gt + 1], None,
                                                ALU.is_equal)
                        nc.tensor.matmul(agg_ps[:], em_sb[:], sj[:],
                                         start=(t == 0), stop=(t == TA - 1))
                    aggT_sb = sm.tile([64, 128], DT.bfloat16, name="aggT_sb")
                    nc.scalar.activation(aggT_sb[:], agg_ps[:], AF.Copy)
                    apj_ps = pw.tile([128, 64], DT.float32, name="apj_ps", tag="w")
                    nc.tensor.matmul(apj_ps[:], ones_b_s[:], b2r_s[:, lw],
                                     start=True, stop=False)
                    nc.tensor.matmul(apj_ps[:], aggT_sb[:], w2b_s[:, lw],
                                     start=False, stop=True)
                    # agg_proj rows padded to 256B for the SBUF-source gather;
                    # pad cols must be finite (zero) since SEL kills them.
                    apj_pad = sm.tile([128, 128], DT.bfloat16, name="apj_pad")
                    nc.vector.memset(apj_pad[:, 64:128], 0.0)
                    nc.scalar.activation(apj_pad[:, 0:64], apj_ps[:], AF.Copy)

                    # ================= stage B: edges -> aggr =================
                    agB = gat.tile([128, 1, CAPB], DT.bfloat16, name="agB")
                    for s in range(0, CAPB, 512):
                        w = min(512, CAPB - s)
                        nc.gpsimd.dma_gather(
                            agB[:, :, s:s + w], table_ap,
                            b_idx_s[:, (ch * CAPB + s) // 16:
                                    (ch * CAPB + s + w) // 16],
                            w, w, 128, transpose=True)
                    # expand agg_proj[dst] via SBUF-source gather (dst_local idxs)
                    gapj = gat.tile([128, 1, CAPB], DT.bfloat16, name="gapj")
                    for s in range(0, CAPB, 512):
                        w = min(512, CAPB - s)
                        nc.gpsimd.dma_gather(
                            gapj[:, :, s:s + w], apj_pad[:],
                            b_dst_s[:, (ch * CAPB + s) // 16:
                                    (ch * CAPB + s + w) // 16],
                            w, w, 128, transpose=True,
                            sbuf_tokens_per_rank=128, sbuf_free_dim_per_rank=256,
                            sbuf_free_dim_pad_per_rank=0, sbuf_byte_offset=0)
                    aggr_ps = pa.tile([64, 128], DT.float32, name="aggr_ps", tag="acc")
                    for t in range(TB):
                        gt = ch * TB + t
                        de = oh.tile([128, 128], DT.bfloat16, name="de")
                        nc.vector.tensor_scalar(de[:], iota_r_s[:],
                                                b_cmp_s[:, gt:gt + 1], None,
                                                ALU.is_equal)
                        fi_ps = pw.tile([128, 64], DT.float32, name="fi_ps", tag="w")
                        nc.tensor.matmul(fi_ps[:], agB[:, 0, t * 128:(t + 1) * 128],
                                         w2a_s[:, lw], start=True, stop=False)
                        nc.tensor.matmul(fi_ps[:], gapj[:, 0, t * 128:(t + 1) * 128],
                                         sel64_s[:], start=False, stop=True)
                        fi_sb = sm.tile([128, 64], DT.bfloat16, name="fi_sb")
                        nc.scalar.activation(fi_sb[:], fi_ps[:], AF.Relu)
                        nc.tensor.matmul(aggr_ps[:], fi_sb[:], de[:],
                                         start=(t == 0), stop=(t == TB - 1))
                    aggr_sb = sm.tile([64, 128], DT.bfloat16, name="aggr_sb")
                    nc.scalar.activation(aggr_sb[:], aggr_ps[:], AF.Copy)

                    # ================= stage C: node update =================
                    chs = slice(ch * 128, (ch + 1) * 128)
                    z_ps = pw.tile([64, 128], DT.float32, name="z_ps", tag="w")
                    nc.tensor.matmul(z_ps[:], wn1a_s[:, lw], hT[:, chs],
                                     start=True, stop=False)
                    nc.tensor.matmul(z_ps[:], wn1b_s[:, lw], aggr_sb[:],
                                     start=False, stop=True)
                    z_sb = sm.tile([64, 128], DT.bfloat16, name="z_sb")
                    nc.scalar.activation(z_sb[:], z_ps[:], AF.Relu,
                                         bias=bn1c_s[:, l:l + 1])
                    hn_ps = pw.tile([128, 64], DT.float32, name="hn_ps", tag="w")
                    nc.tensor.matmul(hn_ps[:], ones_b_s[:], bn2r_s[:, lw],
                                     start=True, stop=False)
                    nc.tensor.matmul(hn_ps[:], z_sb[:], wn2_s[:, lw],
                                     start=False, stop=True)
                    if l < L - 1:
                        hn_pad = sm.tile([128, 128], DT.bfloat16, name="hn_pad")
                        nc.vector.memset(hn_pad[:, 64:128], 0.0)
                        nc.scalar.activation(hn_pad[:, 0:64], hn_ps[:], AF.Copy)
                        nc.sync.dma_start(bounce[l][chs, :], hn_pad[:])
                        hT_ps = pw.tile([64, 128], DT.float32, name="hT_ps", tag="w")
                        nc.tensor.matmul(hT_ps[:], wn2_s[:, lw], z_sb[:],
                                         start=True, stop=True)
                        nc.scalar.activation(hT[:, chs], hT_ps[:], AF.Identity,
                                             bias=bn2c_s[:, l:l + 1])
                    else:
                        hn_sb = sm.tile([128, 64], DT.bfloat16, name="hn_sb")
                        nc.scalar.activation(hn_sb[:], hn_ps[:], AF.Copy)
                        bm = oh.tile([128, 128], DT.bfloat16, name="bm")
                        nc.vector.tensor_scalar(bm[:], iota_r_s[:],
                                                p_cmp_s[:, ch:ch + 1], None,
                                                ALU.is_equal)
                        nc.tensor.matmul(pooled_ps[:], hn_sb[:], bm[:],
                                         start=(ch == 0), stop=(ch == NCH - 1))
                if l < L - 1:
                    nc.gpsimd.collective_compute(
                        "AllGather", ALU.bypass,
                        replica_groups=[list(range(NCORES))],
                        ins=[bounce[l].opt()], outs=[tables[l].opt()])

            # ================= pooling + head =================
            pooled_sb = res.tile([64, 128], DT.float32, name="pooled_sb")
            nc.scalar.activation(pooled_sb[:], pooled_ps[:], AF.Copy)
            nc.sync.dma_start(pb_in[:, :], pooled_sb[:])
            nc.gpsimd.collective_compute(
                "AllReduce", ALU.add, replica_groups=[list(range(NCORES))],
                ins=[pb_in.opt()], outs=[pb_out.opt()])
            pooled_all = res.tile([64, 128], DT.float32, name="pooled_all")
            nc.sync.dma_start(pooled_all[:], pb_out[:, :])
            ones_f = res.tile([1, 64], DT.float32, name="ones_f")
            nc.vector.memset(ones_f[:], 1.0)
            inv_ps = pw.tile([64, 128], DT.float32, name="inv_ps", tag="w")
            nc.tensor.matmul(inv_ps[:], ones_f[:], invr_s[:], start=True, stop=True)
            pm_sb = res.tile([64, 128], DT.float32, name="pm_sb")
            nc.vector.tensor_tensor(pm_sb[:], pooled_all[:], inv_ps[:], ALU.mult)
            q_sb = res.tile([64, 128], DT.float32, name="q_sb")
            nc.scalar.activation(q_sb[:], pm_sb[:], AF.Relu)
            o1_ps = pw.tile([64, 128], DT.float32, name="o1_ps", tag="w")
            nc.tensor.matmul(o1_ps[:], wo1_s[:], q_sb[:], start=True, stop=True)
            o1_sb = res.tile([64, 128], DT.float32, name="o1_sb")
            nc.scalar.activation(o1_sb[:], o1_ps[:], AF.Relu, bias=bo1c_s[:])
            o2_ps = pw.tile([32, 128], DT.float32, name="o2_ps", tag="w")
            nc.tensor.matmul(o2_ps[:], wo2_s[:], o1_sb[:], start=True, stop=True)
            o2_sb = res.tile([32, 128], DT.float32, name="o2_sb")
            nc.scalar.activation(o2_sb[:], o2_ps[:], AF.Identity, bias=bo2c_s[:])
            nc.sync.dma_start(outT.ap(), o2_sb[:])
    nc.compile()
    return nc


def _pack_inputs(inputs):
    x = np.asarray(inputs["x"], F32)
    rbf = np.asarray(inputs["rbf"], F32)
    cbf = np.asarray(inputs["cbf"], F32)
    ei = np.asarray(inputs["edge_index"]).astype(np.int64)
    src, dst = ei[0], ei[1]
    k_idx = np.asarray(inputs["k_idx"]).astype(np.int64)
    j_idx = np.asarray(inputs["j_idx"]).astype(np.int64)
    batch = np.asarray(inputs["batch"]).astype(np.int64)
    W1 = np.asarray(inputs["W1"], F32); b1 = np.asarray(inputs["b1"], F32)
    W2 = np.asarray(inputs["W2"], F32); b2 = np.asarray(inputs["b2"], F32)
    Wn1 = np.asarray(inputs["Wn1"], F32); bn1 = np.asarray(inputs["bn1"], F32)
    Wn2 = np.asarray(inputs["Wn2"], F32); bn2 = np.asarray(inputs["bn2"], F32)
    Wo1 = np.asarray(inputs["Wo1"], F32); bo1 = np.asarray(inputs["bo1"], F32)
    Wo2 = np.asarray(inputs["Wo2"], F32); bo2 = np.asarray(inputs["bo2"], F32)

    # ---- triplet filtering + bucketing by j chunk ----
    keep = j_idx < N
    kk = k_idx[keep]; jk = j_idx[keep]
    rck = np.concatenate([rbf[jk], cbf[keep]], axis=1)   # [Tk, 12]

    gA = jk // 128
    cntA = np.bincount(gA, minlength=NCHG)
    TA = max(2, int(np.ceil(cntA.max() / 128)))
    CAPA = TA * 128
    posA = _bucket(gA, CAPA, len(jk))
    ak = np.zeros(NCHG * CAPA, np.int64); ak[posA] = kk
    acmp = np.full(NCHG * CAPA, -1.0, F32); acmp[posA] = (jk % 128).astype(F32)
    arc = np.zeros((NCHG * CAPA, 12), F32); arc[posA] = rck

    # ---- edge bucketing by dst chunk ----
    gB = dst // 128
    cntB = np.bincount(gB, minlength=NCHG)
    TB = max(2, int(np.ceil(cntB.max() / 128)))
    CAPB = TB * 128
    posB = _bucket(gB, CAPB, E)
    bsrc = np.zeros(NCHG * CAPB, np.int64); bsrc[posB] = src
    bcmp = np.full(NCHG * CAPB, -1.0, F32); bcmp[posB] = (dst % 128).astype(F32)
    bdst = np.zeros(NCHG * CAPB, np.int64); bdst[posB] = dst % 128

    # ---- shared (replicated) tensors ----
    xpad = np.zeros((NPAD, 128), BF16)
    xpad[:N, :64] = x.astype(BF16)
    cnt = np.bincount(batch, minlength=128).astype(F32)[:128]
    invr = (1.0 / np.maximum(cnt, 1.0))[None, :].astype(F32)
    iota_r = np.ascontiguousarray(
        np.broadcast_to(np.arange(128, dtype=F32), (128, 128))).astype(BF16)
    ones_b = np.ones((1, 128), BF16)
    sel64v = np.zeros((128, 64), BF16); sel64v[:64] = np.eye(64, dtype=BF16)

    w1h = np.zeros((L, 128, 64), BF16); w1h[:, :64] = W1[:, :64].astype(BF16)
    w1rc = np.zeros((L, 16, 64), BF16)
    w1rc[:, :12] = W1[:, 64:76].astype(BF16)
    w1rc[:, 12] = b1.astype(BF16)
    w2a = np.zeros((L, 128, 64), BF16); w2a[:, :64] = W2[:, :64].astype(BF16)
    w2b = W2[:, 64:].astype(BF16)
    b2r = b2[:, None, :].astype(BF16)
    wn1a = Wn1[:, :64].astype(BF16)
    wn1b = Wn1[:, 64:].astype(BF16)
    wn2k = Wn2.astype(BF16)
    bn1ck = bn1[:, :, None].astype(F32)
    bn2ck = bn2[:, :, None].astype(F32)
    bn2rk = bn2[:, None, :].astype(BF16)

    batch_pad = np.full(NPAD, -1.0, F32)
    batch_pad[:N] = batch.astype(F32)

    shared = dict(
        xpad=xpad, iota_r=iota_r, ones_b=ones_b, sel64=sel64v,
        w1h=w1h, w1rc=w1rc, w2a=w2a, w2b=w2b, b2r=b2r,
        wn1a=wn1a, wn1b=wn1b, wn2=wn2k, bn1c=bn1ck, bn2c=bn2ck, bn2r=bn2rk,
        wo1=Wo1, bo1c=bo1[:, None].astype(F32), wo2=Wo2,
        bo2c=bo2[:, None].astype(F32), invr=invr,
    )

    in_maps = []
    for c in range(NCORES):
        ts = slice(c * NCH * CAPA, (c + 1) * NCH * CAPA)   # triplet slots
        es = slice(c * NCH * CAPB, (c + 1) * NCH * CAPB)   # edge slots
        ns = slice(c * CN, (c + 1) * CN)                   # node slots
        a_rc_c = np.zeros((16, NCH * CAPA), BF16)
        a_rc_c[:12] = arc[ts].T.astype(BF16)
        a_rc_c[12] = 1.0
        xT_c = np.zeros((64, CN), BF16)
        nhi = min((c + 1) * CN, N)
        if nhi > c * CN:
            xT_c[:, :nhi - c * CN] = x[c * CN:nhi].T.astype(BF16)
        m = dict(shared)
        m.update(
            xT=xT_c,
            a_idx=_wrap_idx(ak[ts]),
            a_cmp=np.ascontiguousarray(
                acmp[ts].reshape(NCH * TA, 128).T),
            a_rc=a_rc_c,
            b_idx=_wrap_idx(bsrc[es]),
            b_cmp=np.ascontiguousarray(
                bcmp[es].reshape(NCH * TB, 128).T),
            b_dst=_wrap_idx(bdst[es]),
            p_cmp=np.ascontiguousarray(
                batch_pad[ns].reshape(NCH, 128).T),
        )
        in_maps.append(m)
    return TA, TB, in_maps


_PROG_CACHE = {}


def kernel(**inputs) -> np.ndarray:
    TA, TB, in_maps = _pack_inputs(inputs)
    key = (TA, TB)
    if key not in _PROG_CACHE:
        _PROG_CACHE[key] = _build_program(TA, TB)
    nc = _PROG_CACHE[key]
    res = run_bass_kernel_spmd(nc, in_maps, core_ids=list(range(NCORES)))
    return np.ascontiguousarray(res.results[0]["outT"].T).astype(F32)


# revision 12
# speedup vs baseline: 1.7405x; 1.6431x over previous
"""DimeNet-style GNN message passing on 8 Trainium2 NeuronCores.

Strategy
--------
Only rows dst<N of the [E,H] triplet-aggregation buffer are ever read
(agg_e[dst] with dst in [0,N)), so triplets with j_idx >= N are dead:
~40k of 640k triplets survive.

Sharding: core c owns node range [2048c, 2048(c+1)).  Triplets are
bucketed by j_idx//128 (node chunk), edges by dst//128.  All segment
sums become chunk-local one-hot matmuls accumulated in PSUM.  The only
collectives are an AllGather of per-node projections after layers 0/1
and a tiny AllReduce of pooled per-graph sums.

The gathered node table holds projections, not h: row n = [h[n]@W2a(l),
h[n]@W1h(l)] (bf16, 256B rows).  Stage A (triplets) is then
relu(hw1[k_idx] + rcproj) where rcproj = [rbf|cbf]@W1rc + b1 is
host-precomputed per layer, and stage B (edges) is
relu(hproj[src] + aggproj[dst]) - both pure elementwise on gathered
rows; no per-tile MLP matmuls.  Gathers use non-transpose dma_gather
(~9ns/idx of GpSimd descriptor generation - the kernel bottleneck).
"""
import sys

if '/opt/trn_rl_repo' not in sys.path:
    sys.path.insert(0, '/opt/trn_rl_repo')

import numpy as np
import ml_dtypes

import concourse.bacc as bacc
import concourse.mybir as mybir
import concourse.tile as tile
from concourse.bass_utils import run_bass_kernel_spmd

BF16 = ml_dtypes.bfloat16
F32 = np.float32

N, E, T, B = 16000, 256000, 640000, 128
FIN, H, OUT, L = 64, 64, 32, 3
NCORES = 8
NCH = 16                 # node chunks per core (128 nodes each)
CN = NCH * 128           # 2048 nodes per core
NPAD = NCORES * CN       # 16384 padded node table rows
NCHG = NCORES * NCH      # 128 global chunks

AF = mybir.ActivationFunctionType
ALU = mybir.AluOpType
DT = mybir.dt


def _wrap_idx(ids: np.ndarray) -> np.ndarray:
    """dma_gather index layout: idx i -> [i%16, i//16], replicated to 128 partitions."""
    s = ids.shape[0]
    w = ids.reshape(s // 16, 16).T.astype(np.int16)
    return np.ascontiguousarray(np.tile(w, (8, 1)))


def _bucket(gchunk: np.ndarray, cap: int, nvals: int):
    """Slot position for each element: gchunk*cap + rank-within-chunk."""
    order = np.argsort(gchunk, kind='stable')
    sg = gchunk[order]
    starts = np.searchsorted(sg, np.arange(NCHG))
    rank = np.arange(len(sg)) - starts[sg]
    assert rank.max() < cap if len(rank) else True
    pos = sg * cap + rank
    out_pos = np.empty(nvals, np.int64)
    out_pos[order] = pos
    return out_pos


def _build_program(TA: int, TB: int):
    CAPA, CAPB = TA * 128, TB * 128
    nc = bacc.Bacc("TRN2", target_bir_lowering=False, debug=False,
                   num_devices=NCORES)

    # ---------------- DRAM I/O ----------------
    def din(name, shape, dt):
        return nc.dram_tensor(name, list(shape), dt, kind="ExternalInput")

    xpad = din("xpad", [NPAD, 128], DT.bfloat16)       # [x@W2a0 | x@W1h0]
    xT = din("xT", [64, CN], DT.bfloat16)
    a_idx = din("a_idx", [128, NCH * CAPA // 16], DT.int16)
    a_cmp = din("a_cmp", [128, NCH * TA], DT.float32)
    a_rcp = din("a_rcp", [L, NCH * CAPA, 64], DT.bfloat16)
    b_idx = din("b_idx", [128, NCH * CAPB // 16], DT.int16)
    b_cmp = din("b_cmp", [128, NCH * TB], DT.float32)
    b_cmprow = din("b_cmprow", [1, NCH * CAPB], DT.bfloat16)
    p_cmp = din("p_cmp", [128, NCH], DT.float32)
    iota_r = din("iota_r", [128, 128], DT.bfloat16)
    iota_c = din("iota_c", [128, 1], DT.float32)
    ones_b = din("ones_b", [1, 128], DT.bfloat16)
    w1h = din("w1h", [L, 64, 64], DT.bfloat16)
    w2a = din("w2a", [L, 64, 64], DT.bfloat16)
    w2b = din("w2b", [L, 64, 64], DT.bfloat16)
    b2r = din("b2r", [L, 1, 64], DT.bfloat16)
    wn1a = din("wn1a", [L, 64, 64], DT.bfloat16)
    wn1b = din("wn1b", [L, 64, 64], DT.bfloat16)
    wn2 = din("wn2", [L, 64, 64], DT.bfloat16)
    bn1c = din("bn1c", [L, 64, 1], DT.float32)
    bn2c = din("bn2c", [L, 64, 1], DT.float32)
    bn2r = din("bn2r", [L, 1, 64], DT.bfloat16)
    wo1 = din("wo1", [64, 64], DT.float32)
    bo1c = din("bo1c", [64, 1], DT.float32)
    wo2 = din("wo2", [64, 32], DT.float32)
    bo2c = din("bo2c", [32, 1], DT.float32)
    invr = din("invr", [1, 128], DT.float32)
    outT = nc.dram_tensor("outT", [32, 128], DT.float32, kind="ExternalOutput")

    with tile.TileContext(nc) as tc:
        with (
            tc.tile_pool(name="res", bufs=1) as res,           # resident SBUF
            tc.tile_pool(name="gat", bufs=3) as gat,           # gather tiles
            tc.tile_pool(name="oh", bufs=4) as oh,             # one-hot tiles
            tc.tile_pool(name="sm", bufs=4) as sm,             # small evacuations
            tc.tile_pool(name="pw", bufs=3, space="PSUM") as pw,    # work psum
            tc.tile_pool(name="pa", bufs=3, space="PSUM") as pa,    # accumulators
            tc.tile_pool(name="pp", bufs=1, space="PSUM") as pp,    # pooled accum
            tc.tile_pool(name="dram", bufs=1, space="DRAM") as dram,
        ):
            # ---------- load resident data ----------
            def load(src, shape, dt, name):
                t = res.tile(list(shape), dt, name=name)
                nc.sync.dma_start(t[:], src)
                return t

            a_idx_s = load(a_idx.ap(), [128, NCH * CAPA // 16], DT.int16, "a_idx_s")
            a_cmp_s = load(a_cmp.ap(), [128, NCH * TA], DT.float32, "a_cmp_s")
            b_idx_s = load(b_idx.ap(), [128, NCH * CAPB // 16], DT.int16, "b_idx_s")
            b_cmp_s = load(b_cmp.ap(), [128, NCH * TB], DT.float32, "b_cmp_s")
            b_cmprow_s = load(b_cmprow.ap(), [1, NCH * CAPB], DT.bfloat16,
                              "b_cmprow_s")
            p_cmp_s = load(p_cmp.ap(), [128, NCH], DT.float32, "p_cmp_s")
            iota_r_s = load(iota_r.ap(), [128, 128], DT.bfloat16, "iota_r_s")
            iota_c_s = load(iota_c.ap(), [128, 1], DT.float32, "iota_c_s")
            ones_b_s = load(ones_b.ap(), [1, 128], DT.bfloat16, "ones_b_s")

            def load_l(src, parts, width, dt, name):
                t = res.tile([parts, L * width], dt, name=name)
                for l in range(L):
                    nc.sync.dma_start(t[:, l * width:(l + 1) * width],
                                      src.ap()[l])
                return t

            w1h_s = load_l(w1h, 64, 64, DT.bfloat16, "w1h_s")
            w2a_s = load_l(w2a, 64, 64, DT.bfloat16, "w2a_s")
            w2b_s = load_l(w2b, 64, 64, DT.bfloat16, "w2b_s")
            b2r_s = load_l(b2r, 1, 64, DT.bfloat16, "b2r_s")
            wn1a_s = load_l(wn1a, 64, 64, DT.bfloat16, "wn1a_s")
            wn1b_s = load_l(wn1b, 64, 64, DT.bfloat16, "wn1b_s")
            wn2_s = load_l(wn2, 64, 64, DT.bfloat16, "wn2_s")
            bn1c_s = load_l(bn1c, 64, 1, DT.float32, "bn1c_s")
            bn2c_s = load_l(bn2c, 64, 1, DT.float32, "bn2c_s")
            bn2r_s = load_l(bn2r, 1, 64, DT.bfloat16, "bn2r_s")
            wo1_s = load(wo1.ap(), [64, 64], DT.float32, "wo1_s")
            bo1c_s = load(bo1c.ap(), [64, 1], DT.float32, "bo1c_s")
            wo2_s = load(wo2.ap(), [64, 32], DT.float32, "wo2_s")
            bo2c_s = load(bo2c.ap(), [32, 1], DT.float32, "bo2c_s")
            invr_s = load(invr.ap(), [1, 128], DT.float32, "invr_s")
            hT = load(xT.ap(), [64, CN], DT.bfloat16, "hT")

            # ---------- internal DRAM ----------
            bounce = [dram.tile([CN, 128], DT.bfloat16, name=f"bounce{l}")
                      for l in range(L - 1)]
            tables = [dram.tile([NPAD, 128], DT.bfloat16, name=f"table{l}")
                      for l in range(1, L)]
            pb_in = dram.tile([64, 128], DT.float32, name="pb_in")
            pb_out = dram.tile([64, 128], DT.float32, name="pb_out")

            pooled_ps = pp.tile([64, 128], DT.float32, name="pooled_ps")

            def gather(pool_name, table_ap, idx_s, base, cap):
                g = gat.tile([128, cap // 128, 128], DT.bfloat16, name=pool_name)
                for s in range(0, cap, 512):
                    w = min(512, cap - s)
                    nc.gpsimd.dma_gather(
                        g[:, s // 128:(s + w) // 128, :], table_ap,
                        idx_s[:, (base + s) // 16:(base + s + w) // 16],
                        w, w, 128, transpose=False)
                return g

            for l in range(L):
                table_ap = xpad.ap() if l == 0 else tables[l - 1][:, :]
                lw = slice(l * 64, (l + 1) * 64)
                lwn = slice((l + 1) * 64, (l + 2) * 64)   # next layer weights
                for ch in range(NCH):
                    # ===== stage A: em = relu(hw1[k] + rcproj); scatter to agg =====
                    agA = gather("agA", table_ap, a_idx_s, ch * CAPA, CAPA)
                    rcp = gat.tile([128, TA, 64], DT.bfloat16, name="rcp")
                    nc.sync.dma_start(
                        rcp[:],
                        a_rcp.ap()[l, ch * CAPA:(ch + 1) * CAPA, :].rearrange(
                            "(t p) f -> p t f", p=128))
                    agg_ps = pa.tile([64, 128], DT.float32, name="agg_ps",
                                     tag="acc")
                    for t in range(TA):
                        gt = ch * TA + t
                        em_sb = sm.tile([128, 64], DT.bfloat16, name="em_sb")
                        nc.vector.tensor_tensor(em_sb[:], agA[:, t, 64:128],
                                                rcp[:, t, :], ALU.add)
                        nc.scalar.activation(em_sb[:], em_sb[:], AF.Relu)
                        sj = oh.tile([128, 128], DT.bfloat16, name="sj")
                        nc.vector.tensor_scalar(sj[:], iota_r_s[:],
                                                a_cmp_s[:, gt:gt + 1], None,
                                                ALU.is_equal)
                        nc.tensor.matmul(agg_ps[:], em_sb[:], sj[:],
                                         start=(t == 0), stop=(t == TA - 1))
                    aggT_sb = sm.tile([64, 128], DT.bfloat16, name="aggT_sb")
                    nc.scalar.activation(aggT_sb[:], agg_ps[:], AF.Copy)
                    apj_ps = pw.tile([128, 64], DT.float32, name="apj_ps", tag="w")
                    nc.tensor.matmul(apj_ps[:], ones_b_s[:], b2r_s[:, lw],
                                     start=True, stop=False)
                    nc.tensor.matmul(apj_ps[:], aggT_sb[:], w2b_s[:, lw],
                                     start=False, stop=True)
                    apj_sb = sm.tile([128, 64], DT.bfloat16, name="apj_sb")
                    nc.scalar.activation(apj_sb[:], apj_ps[:], AF.Copy)

                    # ===== stage B: fi = relu(hproj[src] + apj[dst]); scatter =====
                    agB = gather("agB", table_ap, b_idx_s, ch * CAPB, CAPB)
                    aggr_ps = pa.tile([64, 128], DT.float32, name="aggr_ps",
                                      tag="acc")
                    for t in range(TB):
                        gt = ch * TB + t
                        de = oh.tile([128, 128], DT.bfloat16, name="de")
                        nc.vector.tensor_scalar(de[:], iota_r_s[:],
                                                b_cmp_s[:, gt:gt + 1], None,
                                                ALU.is_equal)
                        bc_ps = pw.tile([128, 128], DT.float32, name="bc_ps",
                                        tag="w")
                        nc.tensor.matmul(bc_ps[:], ones_b_s[:],
                                         b_cmprow_s[:, gt * 128:(gt + 1) * 128],
                                         start=True, stop=True)
                        dt_ = oh.tile([128, 128], DT.bfloat16, name="dt_")
                        nc.vector.tensor_scalar(dt_[:], bc_ps[:], iota_c_s[:],
                                                None, ALU.is_equal)
                        fi_ps = pw.tile([128, 64], DT.float32, name="fi_ps",
                                        tag="w")
                        nc.tensor.matmul(fi_ps[:], dt_[:], apj_sb[:],
                                         start=True, stop=True)
                        fi_sb = sm.tile([128, 64], DT.bfloat16, name="fi_sb")
                        nc.vector.tensor_tensor(fi_sb[:], fi_ps[:],
                                                agB[:, t, 0:64], ALU.add)
                        nc.scalar.activation(fi_sb[:], fi_sb[:], AF.Relu)
                        nc.tensor.matmul(aggr_ps[:], fi_sb[:], de[:],
                                         start=(t == 0), stop=(t == TB - 1))
                    aggr_sb = sm.tile([64, 128], DT.bfloat16, name="aggr_sb")
                    nc.scalar.activation(aggr_sb[:], aggr_ps[:], AF.Copy)

                    # ===== stage C: node update =====
                    chs = slice(ch * 128, (ch + 1) * 128)
                    z_ps = pw.tile([64, 128], DT.float32, name="z_ps", tag="w")
                    nc.tensor.matmul(z_ps[:], wn1a_s[:, lw], hT[:, chs],
                                     start=True, stop=False)
                    nc.tensor.matmul(z_ps[:], wn1b_s[:, lw], aggr_sb[:],
                                     start=False, stop=True)
                    z_sb = sm.tile([64, 128], DT.bfloat16, name="z_sb")
                    nc.scalar.activation(z_sb[:], z_ps[:], AF.Relu,
                                         bias=bn1c_s[:, l:l + 1])
                    hT_ps = pw.tile([64, 128], DT.float32, name="hT_ps", tag="w")
                    nc.tensor.matmul(hT_ps[:], wn2_s[:, lw], z_sb[:],
                                     start=True, stop=True)
                    nc.scalar.activation(hT[:, chs], hT_ps[:], AF.Identity,
                                         bias=bn2c_s[:, l:l + 1])
                    if l < L - 1:
                        # next-layer projections [h@W2a | h@W1h] -> bounce
                        pj_pad = sm.tile([128, 128], DT.bfloat16, name="pj_pad")
                        pj_ps = pw.tile([128, 64], DT.float32, name="pj_ps",
                                        tag="w")
                        nc.tensor.matmul(pj_ps[:], hT[:, chs], w2a_s[:, lwn],
                                         start=True, stop=True)
                        nc.scalar.activation(pj_pad[:, 0:64], pj_ps[:], AF.Copy)
                        pj2_ps = pw.tile([128, 64], DT.float32, name="pj2_ps",
                                         tag="w")
                        nc.tensor.matmul(pj2_ps[:], hT[:, chs], w1h_s[:, lwn],
                                         start=True, stop=True)
                        nc.scalar.activation(pj_pad[:, 64:128], pj2_ps[:],
                                             AF.Copy)
                        nc.sync.dma_start(bounce[l][chs, :], pj_pad[:])
                    else:
                        hn_ps = pw.tile([128, 64], DT.float32, name="hn_ps",
                                        tag="w")
                        nc.tensor.matmul(hn_ps[:], ones_b_s[:], bn2r_s[:, lw],
                                         start=True, stop=False)
                        nc.tensor.matmul(hn_ps[:], z_sb[:], wn2_s[:, lw],
                                         start=False, stop=True)
                        hn_sb = sm.tile([128, 64], DT.bfloat16, name="hn_sb")
                        nc.scalar.activation(hn_sb[:], hn_ps[:], AF.Copy)
                        bm = oh.tile([128, 128], DT.bfloat16, name="bm")
                        nc.vector.tensor_scalar(bm[:], iota_r_s[:],
                                                p_cmp_s[:, ch:ch + 1], None,
                                                ALU.is_equal)
                        nc.tensor.matmul(pooled_ps[:], hn_sb[:], bm[:],
                                         start=(ch == 0), stop=(ch == NCH - 1))
                if l < L - 1:
                    nc.gpsimd.collective_compute(
                        "AllGather", ALU.bypass,
                        replica_groups=[list(range(NCORES))],
                        ins=[bounce[l].opt()], outs=[tables[l].opt()])

            # ================= pooling + head =================
            pooled_sb = res.tile([64, 128], DT.float32, name="pooled_sb")
            nc.scalar.activation(pooled_sb[:], pooled_ps[:], AF.Copy)
            nc.sync.dma_start(pb_in[:, :], pooled_sb[:])
            nc.gpsimd.collective_compute(
                "AllReduce", ALU.add, replica_groups=[list(range(NCORES))],
                ins=[pb_in.opt()], outs=[pb_out.opt()])
            pooled_all = res.tile([64, 128], DT.float32, name="pooled_all")
            nc.sync.dma_start(pooled_all[:], pb_out[:, :])
            ones_f = res.tile([1, 64], DT.float32, name="ones_f")
            nc.vector.memset(ones_f[:], 1.0)
            inv_ps = pw.tile([64, 128], DT.float32, name="inv_ps", tag="w")
            nc.tensor.matmul(inv_ps[:], ones_f[:], invr_s[:], start=True,
                             stop=True)
            pm_sb = res.tile([64, 128], DT.float32, name="pm_sb")
            nc.vector.tensor_tensor(pm_sb[:], pooled_all[:], inv_ps[:], ALU.mult)
            q_sb = res.tile([64, 128], DT.float32, name="q_sb")
            nc.scalar.activation(q_sb[:], pm_sb[:], AF.Relu)
            o1_ps = pw.tile([64, 128], DT.float32, name="o1_ps", tag="w")
            nc.tensor.matmul(o1_ps[:], wo1_s[:], q_sb[:], start=True, stop=True)
            o1_sb = res.tile([64, 128], DT.float32, name="o1_sb")
            nc.scalar.activation(o1_sb[:], o1_ps[:], AF.Relu, bias=bo1c_s[:])
            o2_ps = pw.tile([32, 128], DT.float32, name="o2_ps", tag="w")
            nc.tensor.matmul(o2_ps[:], wo2_s[:], o1_sb[:], start=True, stop=True)
            o2_sb = res.tile([32, 128], DT.float32, name="o2_sb")
            nc.scalar.activation(o2_sb[:], o2_ps[:], AF.Identity, bias=bo2c_s[:])
            nc.sync.dma_start(outT.ap(), o2_sb[:])
    nc.compile()
    return nc


def _pack_inputs(inputs):
    x = np.asarray(inputs["x"], F32)
    rbf = np.asarray(inputs["rbf"], F32)
    cbf = np.asarray(inputs["cbf"], F32)
    ei = np.asarray(inputs["edge_index"]).astype(np.int64)
    src, dst = ei[0], ei[1]
    k_idx = np.asarray(inputs["k_idx"]).astype(np.int64)
    j_idx = np.asarray(inputs["j_idx"]).astype(np.int64)
    batch = np.asarray(inputs["batch"]).astype(np.int64)
    W1 = np.asarray(inputs["W1"], F32); b1 = np.asarray(inputs["b1"], F32)
    W2 = np.asarray(inputs["W2"], F32); b2 = np.asarray(inputs["b2"], F32)
    Wn1 = np.asarray(inputs["Wn1"], F32); bn1 = np.asarray(inputs["bn1"], F32)
    Wn2 = np.asarray(inputs["Wn2"], F32); bn2 = np.asarray(inputs["bn2"], F32)
    Wo1 = np.asarray(inputs["Wo1"], F32); bo1 = np.asarray(inputs["bo1"], F32)
    Wo2 = np.asarray(inputs["Wo2"], F32); bo2 = np.asarray(inputs["bo2"], F32)

    # ---- triplet filtering + bucketing by j chunk ----
    keep = j_idx < N
    kk = k_idx[keep]; jk = j_idx[keep]
    rc = np.concatenate([rbf[jk], cbf[keep]], axis=1)   # [Tk, 12] f32

    gA = jk // 128
    cntA = np.bincount(gA, minlength=NCHG)
    TA = max(2, int(np.ceil(cntA.max() / 128)))
    CAPA = TA * 128
    posA = _bucket(gA, CAPA, len(jk))
    ak = np.zeros(NCHG * CAPA, np.int64); ak[posA] = kk
    acmp = np.full(NCHG * CAPA, -1.0, F32); acmp[posA] = (jk % 128).astype(F32)
    # rcproj[l] = [rbf[j]|cbf] @ W1[l][64:76] + b1[l]  (host, layer-dependent)
    rcp = np.zeros((L, NCHG * CAPA, 64), BF16)
    for l in range(L):
        rcp[l][posA] = (rc @ W1[l, 64:76] + b1[l]).astype(BF16)

    # ---- edge bucketing by dst chunk ----
    gB = dst // 128
    cntB = np.bincount(gB, minlength=NCHG)
    TB = max(2, int(np.ceil(cntB.max() / 128)))
    CAPB = TB * 128
    posB = _bucket(gB, CAPB, E)
    bsrc = np.zeros(NCHG * CAPB, np.int64); bsrc[posB] = src
    bcmp = np.full(NCHG * CAPB, -1.0, F32); bcmp[posB] = (dst % 128).astype(F32)

    # ---- shared (replicated) tensors ----
    # layer-0 table: [x@W2a0 | x@W1h0], quantization path matches the
    # device (bf16 operands, f32 accumulate, bf16 store)
    xpad = np.zeros((NPAD, 128), BF16)
    xq = x.astype(BF16).astype(F32)
    xpad[:N, 0:64] = (xq @ W2[0, :64].astype(BF16).astype(F32)).astype(BF16)
    xpad[:N, 64:128] = (xq @ W1[0, :64].astype(BF16).astype(F32)).astype(BF16)
    cnt = np.bincount(batch, minlength=128).astype(F32)[:128]
    invr = (1.0 / np.maximum(cnt, 1.0))[None, :].astype(F32)
    iota_r = np.ascontiguousarray(
        np.broadcast_to(np.arange(128, dtype=F32), (128, 128))).astype(BF16)
    iota_c = np.arange(128, dtype=F32)[:, None]
    ones_b = np.ones((1, 128), BF16)

    batch_pad = np.full(NPAD, -1.0, F32)
    batch_pad[:N] = batch.astype(F32)

    shared = dict(
        xpad=xpad, iota_r=iota_r, iota_c=iota_c, ones_b=ones_b,
        w1h=W1[:, :64].astype(BF16), w2a=W2[:, :64].astype(BF16),
        w2b=W2[:, 64:].astype(BF16), b2r=b2[:, None, :].astype(BF16),
        wn1a=Wn1[:, :64].astype(BF16), wn1b=Wn1[:, 64:].astype(BF16),
        wn2=Wn2.astype(BF16),
        bn1c=bn1[:, :, None].astype(F32), bn2c=bn2[:, :, None].astype(F32),
        bn2r=bn2[:, None, :].astype(BF16),
        wo1=Wo1, bo1c=bo1[:, None].astype(F32), wo2=Wo2,
        bo2c=bo2[:, None].astype(F32), invr=invr,
    )

    in_maps = []
    for c in range(NCORES):
        ts = slice(c * NCH * CAPA, (c + 1) * NCH * CAPA)   # triplet slots
        es = slice(c * NCH * CAPB, (c + 1) * NCH * CAPB)   # edge slots
        ns = slice(c * CN, (c + 1) * CN)                   # node slots
        xT_c = np.zeros((64, CN), BF16)
        nhi = min((c + 1) * CN, N)
        if nhi > c * CN:
            xT_c[:, :nhi - c * CN] = x[c * CN:nhi].T.astype(BF16)
        m = dict(shared)
        m.update(
            xT=xT_c,
            a_idx=_wrap_idx(ak[ts]),
            a_cmp=np.ascontiguousarray(
                acmp[ts].reshape(NCH * TA, 128).T),
            a_rcp=np.ascontiguousarray(rcp[:, ts]),
            b_idx=_wrap_idx(bsrc[es]),
            b_cmp=np.ascontiguousarray(
                bcmp[es].reshape(NCH * TB, 128).T),
            b_cmprow=bcmp[es][None, :].astype(BF16),
            p_cmp=np.ascontiguousarray(
                batch_pad[ns].reshape(NCH, 128).T),
        )
        in_maps.append(m)
    return TA, TB, in_maps


_PROG_CACHE = {}


def kernel(**inputs) -> np.ndarray:
    TA, TB, in_maps = _pack_inputs(inputs)
    key = (TA, TB)
    if key not in _PROG_CACHE:
        _PROG_CACHE[key] = _build_program(TA, TB)
    nc = _PROG_CACHE[key]
    res = run_bass_kernel_spmd(nc, in_maps, core_ids=list(range(NCORES)))
    return np.ascontiguousarray(res.results[0]["outT"].T).astype(F32)


# revision 15
# speedup vs baseline: 1.9757x; 1.1351x over previous
"""DimeNet-style GNN message passing on 8 Trainium2 NeuronCores.

Strategy
--------
Only rows dst<N of the [E,H] triplet-aggregation buffer are ever read
(agg_e[dst] with dst in [0,N)), so triplets with j_idx >= N are dead:
~40k of 640k triplets survive.

Sharding: core c owns node range [2048c, 2048(c+1)).  Triplets are
bucketed by j_idx//128 (node chunk), edges by dst//128.  All segment
sums become chunk-local one-hot matmuls accumulated in PSUM.  The only
collectives are an AllGather of per-node projections after layers 0/1
and a tiny AllReduce of pooled per-graph sums.

The gathered node table holds projections, not h: row n = [h[n]@W2a(l),
h[n]@W1h(l)] (bf16, 256B rows).  Stage A (triplets) is then
relu(hw1[k_idx] + rcproj) where rcproj = [rbf|cbf]@W1rc + b1 is
host-precomputed per layer, and stage B (edges) is
relu(hproj[src] + aggproj[dst]) - both pure elementwise on gathered
rows; no per-tile MLP matmuls.  Gathers use non-transpose dma_gather
(~9ns/idx of GpSimd descriptor generation - the kernel bottleneck).
"""
import sys

if '/opt/trn_rl_repo' not in sys.path:
    sys.path.insert(0, '/opt/trn_rl_repo')

import numpy as np
import ml_dtypes

import concourse.bacc as bacc
import concourse.bass as bass
import concourse.mybir as mybir
import concourse.tile as tile
from concourse.bass_utils import run_bass_kernel_spmd

BF16 = ml_dtypes.bfloat16
F32 = np.float32

N, E, T, B = 16000, 256000, 640000, 128
FIN, H, OUT, L = 64, 64, 32, 3
NCORES = 8
NCH = 16                 # node chunks per core (128 nodes each)
CN = NCH * 128           # 2048 nodes per core
NPAD = NCORES * CN       # 16384 padded node table rows
NCHG = NCORES * NCH      # 128 global chunks

AF = mybir.ActivationFunctionType
ALU = mybir.AluOpType
DT = mybir.dt


def _wrap_idx(ids: np.ndarray) -> np.ndarray:
    """dma_gather index layout: idx i -> [i%16, i//16], replicated to 128 partitions."""
    s = ids.shape[0]
    w = ids.reshape(s // 16, 16).T.astype(np.int16)
    return np.ascontiguousarray(np.tile(w, (8, 1)))


def _bucket(gchunk: np.ndarray, cap: int, nvals: int):
    """Slot position for each element: gchunk*cap + rank-within-chunk."""
    order = np.argsort(gchunk, kind='stable')
    sg = gchunk[order]
    starts = np.searchsorted(sg, np.arange(NCHG))
    rank = np.arange(len(sg)) - starts[sg]
    assert rank.max() < cap if len(rank) else True
    pos = sg * cap + rank
    out_pos = np.empty(nvals, np.int64)
    out_pos[order] = pos
    return out_pos


def _build_program(TA: int, TB: int):
    CAPA, CAPB = TA * 128, TB * 128
    nc = bacc.Bacc("TRN2", target_bir_lowering=False, debug=False,
                   num_devices=NCORES)

    # ---------------- DRAM I/O ----------------
    def din(name, shape, dt):
        return nc.dram_tensor(name, list(shape), dt, kind="ExternalInput")

    xpad = din("xpad", [NPAD, 128], DT.bfloat16)       # [x@W2a0 | x@W1h0]
    xT = din("xT", [64, CN], DT.bfloat16)
    a_idx = din("a_idx", [128, NCH * CAPA // 16], DT.int16)
    a_cmp = din("a_cmp", [128, NCH * TA], DT.bfloat16)
    a_rcp = din("a_rcp", [L, NCH * CAPA, 64], DT.bfloat16)
    b_idx = din("b_idx", [128, NCH * CAPB // 16], DT.int16)
    b_cmp = din("b_cmp", [128, NCH * TB], DT.bfloat16)
    b_cmprow = din("b_cmprow", [1, NCH * CAPB], DT.bfloat16)
    p_cmp = din("p_cmp", [128, NCH], DT.bfloat16)
    iota_r = din("iota_r", [128, 128], DT.bfloat16)
    iota_c = din("iota_c", [128, 1], DT.float32)
    ones_b = din("ones_b", [1, 128], DT.bfloat16)
    w1h = din("w1h", [L, 64, 64], DT.bfloat16)
    w2a = din("w2a", [L, 64, 64], DT.bfloat16)
    w2b = din("w2b", [L, 64, 64], DT.bfloat16)
    b2r = din("b2r", [L, 1, 64], DT.bfloat16)
    wn1a = din("wn1a", [L, 64, 64], DT.bfloat16)
    wn1b = din("wn1b", [L, 64, 64], DT.bfloat16)
    wn2 = din("wn2", [L, 64, 64], DT.bfloat16)
    bn1c = din("bn1c", [L, 64, 1], DT.float32)
    bn2c = din("bn2c", [L, 64, 1], DT.float32)
    bn2r = din("bn2r", [L, 1, 64], DT.bfloat16)
    wo1 = din("wo1", [64, 64], DT.float32)
    bo1c = din("bo1c", [64, 1], DT.float32)
    wo2 = din("wo2", [64, 32], DT.float32)
    bo2c = din("bo2c", [32, 1], DT.float32)
    invr = din("invr", [1, 128], DT.float32)
    outT = nc.dram_tensor("outT", [32, 128], DT.float32, kind="ExternalOutput")

    with tile.TileContext(nc) as tc:
        with (
            tc.tile_pool(name="res", bufs=1) as res,           # resident SBUF
            tc.tile_pool(name="gat", bufs=3) as gat,           # gather tiles
            tc.tile_pool(name="oh", bufs=4) as oh,             # one-hot tiles
            tc.tile_pool(name="sm", bufs=4) as sm,             # small evacuations
            tc.tile_pool(name="pw", bufs=3, space="PSUM") as pw,    # work psum
            tc.tile_pool(name="bc", bufs=2, space="PSUM") as bc,    # bcast psum
            tc.tile_pool(name="pa", bufs=2, space="PSUM") as pa,    # accumulators
            tc.tile_pool(name="pp", bufs=1, space="PSUM") as pp,    # pooled accum
            tc.tile_pool(name="dram", bufs=1, space="DRAM") as dram,
        ):
            # ---------- load resident data ----------
            def load(src, shape, dt, name):
                t = res.tile(list(shape), dt, name=name)
                nc.sync.dma_start(t[:], src)
                return t

            a_idx_s = load(a_idx.ap(), [128, NCH * CAPA // 16], DT.int16, "a_idx_s")
            a_cmp_s = load(a_cmp.ap(), [128, NCH * TA], DT.bfloat16, "a_cmp_s")
            b_idx_s = load(b_idx.ap(), [128, NCH * CAPB // 16], DT.int16, "b_idx_s")
            b_cmp_s = load(b_cmp.ap(), [128, NCH * TB], DT.bfloat16, "b_cmp_s")
            b_cmprow_s = load(b_cmprow.ap(), [1, NCH * CAPB], DT.bfloat16,
                              "b_cmprow_s")
            p_cmp_s = load(p_cmp.ap(), [128, NCH], DT.bfloat16, "p_cmp_s")
            iota_r_s = load(iota_r.ap(), [128, 128], DT.bfloat16, "iota_r_s")
            iota_c_s = load(iota_c.ap(), [128, 1], DT.float32, "iota_c_s")
            ones_b_s = load(ones_b.ap(), [1, 128], DT.bfloat16, "ones_b_s")

            def load_l(src, parts, width, dt, name):
                t = res.tile([parts, L * width], dt, name=name)
                for l in range(L):
                    nc.sync.dma_start(t[:, l * width:(l + 1) * width],
                                      src.ap()[l])
                return t

            w1h_s = load_l(w1h, 64, 64, DT.bfloat16, "w1h_s")
            w2a_s = load_l(w2a, 64, 64, DT.bfloat16, "w2a_s")
            w2b_s = load_l(w2b, 64, 64, DT.bfloat16, "w2b_s")
            b2r_s = load_l(b2r, 1, 64, DT.bfloat16, "b2r_s")
            wn1a_s = load_l(wn1a, 64, 64, DT.bfloat16, "wn1a_s")
            wn1b_s = load_l(wn1b, 64, 64, DT.bfloat16, "wn1b_s")
            wn2_s = load_l(wn2, 64, 64, DT.bfloat16, "wn2_s")
            bn1c_s = load_l(bn1c, 64, 1, DT.float32, "bn1c_s")
            bn2c_s = load_l(bn2c, 64, 1, DT.float32, "bn2c_s")
            bn2r_s = load_l(bn2r, 1, 64, DT.bfloat16, "bn2r_s")
            wo1_s = load(wo1.ap(), [64, 64], DT.float32, "wo1_s")
            bo1c_s = load(bo1c.ap(), [64, 1], DT.float32, "bo1c_s")
            wo2_s = load(wo2.ap(), [64, 32], DT.float32, "wo2_s")
            bo2c_s = load(bo2c.ap(), [32, 1], DT.float32, "bo2c_s")
            invr_s = load(invr.ap(), [1, 128], DT.float32, "invr_s")
            hT = load(xT.ap(), [64, CN], DT.bfloat16, "hT")

            # ---------- internal DRAM ----------
            bounce = [dram.tile([CN, 128], DT.bfloat16, name=f"bounce{l}")
                      for l in range(L - 1)]
            tables = [dram.tile([NPAD, 128], DT.bfloat16, name=f"table{l}")
                      for l in range(1, L)]
            pb_in = dram.tile([64, 128], DT.float32, name="pb_in")
            pb_out = dram.tile([64, 128], DT.float32, name="pb_out")

            pooled_ps = pp.tile([64, 128], DT.float32, name="pooled_ps")

            def iota_mid(gw):
                ap = iota_r_s[:]
                return bass.AP(ap.tensor, ap.offset,
                               [ap.ap[0], [0, gw], ap.ap[1]])

            def gather(pool_name, table_ap, idx_s, base, cap):
                g = gat.tile([128, cap // 128, 128], DT.bfloat16, name=pool_name)
                for s in range(0, cap, 512):
                    w = min(512, cap - s)
                    nc.gpsimd.dma_gather(
                        g[:, s // 128:(s + w) // 128, :], table_ap,
                        idx_s[:, (base + s) // 16:(base + s + w) // 16],
                        w, w, 128, transpose=False)
                return g

            for l in range(L):
                table_ap = xpad.ap() if l == 0 else tables[l - 1][:, :]
                lw = slice(l * 64, (l + 1) * 64)
                lwn = slice((l + 1) * 64, (l + 2) * 64)   # next layer weights
                for ch in range(NCH):
                    # ===== stage A: em = relu(hw1[k] + rcproj); scatter to agg =====
                    agA = gather("agA", table_ap, a_idx_s, ch * CAPA, CAPA)
                    rcp = gat.tile([128, TA, 64], DT.bfloat16, name="rcp")
                    nc.sync.dma_start(
                        rcp[:],
                        a_rcp.ap()[l, ch * CAPA:(ch + 1) * CAPA, :].rearrange(
                            "(t p) f -> p t f", p=128))
                    agg_ps = pa.tile([64, 128], DT.float32, name="agg_ps",
                                     tag="acc")
                    # batched: one add + one relu + one one-hot build per chunk
                    em_sb = sm.tile([128, TA, 64], DT.bfloat16, name="em_sb")
                    nc.vector.tensor_tensor(em_sb[:], agA[:, :, 64:128],
                                            rcp[:], ALU.add)
                    nc.scalar.activation(em_sb[:], em_sb[:], AF.Relu)
                    sj = oh.tile([128, TA, 128], DT.bfloat16, name="sj")
                    nc.vector.tensor_tensor(
                        sj[:], iota_mid(TA),
                        a_cmp_s[:, ch * TA:(ch + 1) * TA].to_broadcast(
                            [128, TA, 128]), ALU.is_equal)
                    for t in range(TA):
                        nc.tensor.matmul(agg_ps[:], em_sb[:, t, :], sj[:, t, :],
                                         start=(t == 0), stop=(t == TA - 1))
                    aggT_sb = sm.tile([64, 128], DT.bfloat16, name="aggT_sb")
                    nc.scalar.activation(aggT_sb[:], agg_ps[:], AF.Copy)
                    apj_ps = pw.tile([128, 64], DT.float32, name="apj_ps", tag="w")
                    nc.tensor.matmul(apj_ps[:], ones_b_s[:], b2r_s[:, lw],
                                     start=True, stop=False)
                    nc.tensor.matmul(apj_ps[:], aggT_sb[:], w2b_s[:, lw],
                                     start=False, stop=True)
                    apj_sb = sm.tile([128, 64], DT.bfloat16, name="apj_sb")
                    nc.scalar.activation(apj_sb[:], apj_ps[:], AF.Copy)

                    # ===== stage B: fi = relu(hproj[src] + apj[dst]); scatter =====
                    agB = gather("agB", table_ap, b_idx_s, ch * CAPB, CAPB)
                    aggr_ps = pa.tile([64, 128], DT.float32, name="aggr_ps",
                                      tag="acc")
                    for g0 in range(0, TB, 4):
                        gw = min(4, TB - g0)
                        gt0 = ch * TB + g0
                        de4 = oh.tile([128, 4, 128], DT.bfloat16, name="de4")
                        nc.vector.tensor_tensor(
                            de4[:, :gw, :], iota_mid(gw),
                            b_cmp_s[:, gt0:gt0 + gw].to_broadcast(
                                [128, gw, 128]), ALU.is_equal)
                        bc_ps = bc.tile([128, 512], DT.float32, name="bc_ps")
                        nc.tensor.matmul(
                            bc_ps[:, :gw * 128], ones_b_s[:],
                            b_cmprow_s[:, gt0 * 128:(gt0 + gw) * 128],
                            start=True, stop=True)
                        dt4 = oh.tile([128, 4, 128], DT.bfloat16, name="dt4")
                        nc.vector.tensor_tensor(
                            dt4[:, :gw, :].rearrange("p t e -> p (t e)"),
                            bc_ps[:, :gw * 128],
                            iota_c_s[:].to_broadcast([128, gw * 128]),
                            ALU.is_equal)
                        fi_ps = pw.tile([128, 4, 64], DT.float32, name="fi_ps",
                                        tag="w")
                        for t in range(gw):
                            nc.tensor.matmul(fi_ps[:, t, :], dt4[:, t, :],
                                             apj_sb[:], start=True, stop=True)
                        fi_sb = sm.tile([128, 4, 64], DT.bfloat16, name="fi_sb")
                        nc.vector.tensor_tensor(fi_sb[:, :gw, :],
                                                fi_ps[:, :gw, :],
                                                agB[:, g0:g0 + gw, 0:64],
                                                ALU.add)
                        nc.scalar.activation(
                            fi_sb[:, :gw, :].rearrange("p t e -> p (t e)"),
                            fi_sb[:, :gw, :].rearrange("p t e -> p (t e)"),
                            AF.Relu)
                        for t in range(gw):
                            tt = g0 + t
                            nc.tensor.matmul(aggr_ps[:], fi_sb[:, t, :],
                                             de4[:, t, :],
                                             start=(tt == 0),
                                             stop=(tt == TB - 1))
                    aggr_sb = sm.tile([64, 128], DT.bfloat16, name="aggr_sb")
                    nc.scalar.activation(aggr_sb[:], aggr_ps[:], AF.Copy)

                    # ===== stage C: node update =====
                    chs = slice(ch * 128, (ch + 1) * 128)
                    z_ps = pw.tile([64, 128], DT.float32, name="z_ps", tag="w")
                    nc.tensor.matmul(z_ps[:], wn1a_s[:, lw], hT[:, chs],
                                     start=True, stop=False)
                    nc.tensor.matmul(z_ps[:], wn1b_s[:, lw], aggr_sb[:],
                                     start=False, stop=True)
                    z_sb = sm.tile([64, 128], DT.bfloat16, name="z_sb")
                    nc.scalar.activation(z_sb[:], z_ps[:], AF.Relu,
                                         bias=bn1c_s[:, l:l + 1])
                    hT_ps = pw.tile([64, 128], DT.float32, name="hT_ps", tag="w")
                    nc.tensor.matmul(hT_ps[:], wn2_s[:, lw], z_sb[:],
                                     start=True, stop=True)
                    nc.scalar.activation(hT[:, chs], hT_ps[:], AF.Identity,
                                         bias=bn2c_s[:, l:l + 1])
                    if l < L - 1:
                        # next-layer projections [h@W2a | h@W1h] -> bounce
                        pj_pad = sm.tile([128, 128], DT.bfloat16, name="pj_pad")
                        pj_ps = pw.tile([128, 64], DT.float32, name="pj_ps",
                                        tag="w")
                        nc.tensor.matmul(pj_ps[:], hT[:, chs], w2a_s[:, lwn],
                                         start=True, stop=True)
                        nc.scalar.activation(pj_pad[:, 0:64], pj_ps[:], AF.Copy)
                        pj2_ps = pw.tile([128, 64], DT.float32, name="pj2_ps",
                                         tag="w")
                        nc.tensor.matmul(pj2_ps[:], hT[:, chs], w1h_s[:, lwn],
                                         start=True, stop=True)
                        nc.scalar.activation(pj_pad[:, 64:128], pj2_ps[:],
                                             AF.Copy)
                        nc.sync.dma_start(bounce[l][chs, :], pj_pad[:])
                    else:
                        hn_ps = pw.tile([128, 64], DT.float32, name="hn_ps",
                                        tag="w")
                        nc.tensor.matmul(hn_ps[:], ones_b_s[:], bn2r_s[:, lw],
                                         start=True, stop=False)
                        nc.tensor.matmul(hn_ps[:], z_sb[:], wn2_s[:, lw],
                                         start=False, stop=True)
                        hn_sb = sm.tile([128, 64], DT.bfloat16, name="hn_sb")
                        nc.scalar.activation(hn_sb[:], hn_ps[:], AF.Copy)
                        bm = oh.tile([128, 128], DT.bfloat16, name="bm")
                        nc.vector.tensor_tensor(
                            bm[:], iota_r_s[:],
                            p_cmp_s[:, ch:ch + 1].to_broadcast([128, 128]),
                            ALU.is_equal)
                        nc.tensor.matmul(pooled_ps[:], hn_sb[:], bm[:],
                                         start=(ch == 0), stop=(ch == NCH - 1))
                if l < L - 1:
                    nc.gpsimd.collective_compute(
                        "AllGather", ALU.bypass,
                        replica_groups=[list(range(NCORES))],
                        ins=[bounce[l].opt()], outs=[tables[l].opt()])

            # ================= pooling + head =================
            pooled_sb = res.tile([64, 128], DT.float32, name="pooled_sb")
            nc.scalar.activation(pooled_sb[:], pooled_ps[:], AF.Copy)
            nc.sync.dma_start(pb_in[:, :], pooled_sb[:])
            nc.gpsimd.collective_compute(
                "AllReduce", ALU.add, replica_groups=[list(range(NCORES))],
                ins=[pb_in.opt()], outs=[pb_out.opt()])
            pooled_all = res.tile([64, 128], DT.float32, name="pooled_all")
            nc.sync.dma_start(pooled_all[:], pb_out[:, :])
            ones_f = res.tile([1, 64], DT.float32, name="ones_f")
            nc.vector.memset(ones_f[:], 1.0)
            inv_ps = pw.tile([64, 128], DT.float32, name="inv_ps", tag="w")
            nc.tensor.matmul(inv_ps[:], ones_f[:], invr_s[:], start=True,
                             stop=True)
            pm_sb = res.tile([64, 128], DT.float32, name="pm_sb")
            nc.vector.tensor_tensor(pm_sb[:], pooled_all[:], inv_ps[:], ALU.mult)
            q_sb = res.tile([64, 128], DT.float32, name="q_sb")
            nc.scalar.activation(q_sb[:], pm_sb[:], AF.Relu)
            o1_ps = pw.tile([64, 128], DT.float32, name="o1_ps", tag="w")
            nc.tensor.matmul(o1_ps[:], wo1_s[:], q_sb[:], start=True, stop=True)
            o1_sb = res.tile([64, 128], DT.float32, name="o1_sb")
            nc.scalar.activation(o1_sb[:], o1_ps[:], AF.Relu, bias=bo1c_s[:])
            o2_ps = pw.tile([32, 128], DT.float32, name="o2_ps", tag="w")
            nc.tensor.matmul(o2_ps[:], wo2_s[:], o1_sb[:], start=True, stop=True)
            o2_sb = res.tile([32, 128], DT.float32, name="o2_sb")
            nc.scalar.activation(o2_sb[:], o2_ps[:], AF.Identity, bias=bo2c_s[:])
            nc.sync.dma_start(outT.ap(), o2_sb[:])
    nc.compile()
    return nc


def _pack_inputs(inputs):
    x = np.asarray(inputs["x"], F32)
    rbf = np.asarray(inputs["rbf"], F32)
    cbf = np.asarray(inputs["cbf"], F32)
    ei = np.asarray(inputs["edge_index"]).astype(np.int64)
    src, dst = ei[0], ei[1]
    k_idx = np.asarray(inputs["k_idx"]).astype(np.int64)
    j_idx = np.asarray(inputs["j_idx"]).astype(np.int64)
    batch = np.asarray(inputs["batch"]).astype(np.int64)
    W1 = np.asarray(inputs["W1"], F32); b1 = np.asarray(inputs["b1"], F32)
    W2 = np.asarray(inputs["W2"], F32); b2 = np.asarray(inputs["b2"], F32)
    Wn1 = np.asarray(inputs["Wn1"], F32); bn1 = np.asarray(inputs["bn1"], F32)
    Wn2 = np.asarray(inputs["Wn2"], F32); bn2 = np.asarray(inputs["bn2"], F32)
    Wo1 = np.asarray(inputs["Wo1"], F32); bo1 = np.asarray(inputs["bo1"], F32)
    Wo2 = np.asarray(inputs["Wo2"], F32); bo2 = np.asarray(inputs["bo2"], F32)

    # ---- triplet filtering + bucketing by j chunk ----
    keep = j_idx < N
    kk = k_idx[keep]; jk = j_idx[keep]
    rc = np.concatenate([rbf[jk], cbf[keep]], axis=1)   # [Tk, 12] f32

    gA = jk // 128
    cntA = np.bincount(gA, minlength=NCHG)
    TA = max(2, int(np.ceil(cntA.max() / 128)))
    CAPA = TA * 128
    posA = _bucket(gA, CAPA, len(jk))
    ak = np.zeros(NCHG * CAPA, np.int64); ak[posA] = kk
    acmp = np.full(NCHG * CAPA, -1.0, F32); acmp[posA] = (jk % 128).astype(F32)
    # rcproj[l] = [rbf[j]|cbf] @ W1[l][64:76] + b1[l]  (host, layer-dependent)
    rcp = np.zeros((L, NCHG * CAPA, 64), BF16)
    for l in range(L):
        rcp[l][posA] = (rc @ W1[l, 64:76] + b1[l]).astype(BF16)

    # ---- edge bucketing by dst chunk ----
    gB = dst // 128
    cntB = np.bincount(gB, minlength=NCHG)
    TB = max(2, int(np.ceil(cntB.max() / 128)))
    CAPB = TB * 128
    posB = _bucket(gB, CAPB, E)
    bsrc = np.zeros(NCHG * CAPB, np.int64); bsrc[posB] = src
    bcmp = np.full(NCHG * CAPB, -1.0, F32); bcmp[posB] = (dst % 128).astype(F32)

    # ---- shared (replicated) tensors ----
    # layer-0 table: [x@W2a0 | x@W1h0], quantization path matches the
    # device (bf16 operands, f32 accumulate, bf16 store)
    xpad = np.zeros((NPAD, 128), BF16)
    xq = x.astype(BF16).astype(F32)
    xpad[:N, 0:64] = (xq @ W2[0, :64].astype(BF16).astype(F32)).astype(BF16)
    xpad[:N, 64:128] = (xq @ W1[0, :64].astype(BF16).astype(F32)).astype(BF16)
    cnt = np.bincount(batch, minlength=128).astype(F32)[:128]
    invr = (1.0 / np.maximum(cnt, 1.0))[None, :].astype(F32)
    iota_r = np.ascontiguousarray(
        np.broadcast_to(np.arange(128, dtype=F32), (128, 128))).astype(BF16)
    iota_c = np.arange(128, dtype=F32)[:, None]
    ones_b = np.ones((1, 128), BF16)

    batch_pad = np.full(NPAD, -1.0, F32)
    batch_pad[:N] = batch.astype(F32)

    shared = dict(
        xpad=xpad, iota_r=iota_r, iota_c=iota_c, ones_b=ones_b,
        w1h=W1[:, :64].astype(BF16), w2a=W2[:, :64].astype(BF16),
        w2b=W2[:, 64:].astype(BF16), b2r=b2[:, None, :].astype(BF16),
        wn1a=Wn1[:, :64].astype(BF16), wn1b=Wn1[:, 64:].astype(BF16),
        wn2=Wn2.astype(BF16),
        bn1c=bn1[:, :, None].astype(F32), bn2c=bn2[:, :, None].astype(F32),
        bn2r=bn2[:, None, :].astype(BF16),
        wo1=Wo1, bo1c=bo1[:, None].astype(F32), wo2=Wo2,
        bo2c=bo2[:, None].astype(F32), invr=invr,
    )

    in_maps = []
    for c in range(NCORES):
        ts = slice(c * NCH * CAPA, (c + 1) * NCH * CAPA)   # triplet slots
        es = slice(c * NCH * CAPB, (c + 1) * NCH * CAPB)   # edge slots
        ns = slice(c * CN, (c + 1) * CN)                   # node slots
        xT_c = np.zeros((64, CN), BF16)
        nhi = min((c + 1) * CN, N)
        if nhi > c * CN:
            xT_c[:, :nhi - c * CN] = x[c * CN:nhi].T.astype(BF16)
        m = dict(shared)
        m.update(
            xT=xT_c,
            a_idx=_wrap_idx(ak[ts]),
            a_cmp=np.ascontiguousarray(
                acmp[ts].reshape(NCH * TA, 128).T).astype(BF16),
            a_rcp=np.ascontiguousarray(rcp[:, ts]),
            b_idx=_wrap_idx(bsrc[es]),
            b_cmp=np.ascontiguousarray(
                bcmp[es].reshape(NCH * TB, 128).T).astype(BF16),
            b_cmprow=bcmp[es][None, :].astype(BF16),
            p_cmp=np.ascontiguousarray(
                batch_pad[ns].reshape(NCH, 128).T).astype(BF16),
        )
        in_maps.append(m)
    return TA, TB, in_maps


_PROG_CACHE = {}


def kernel(**inputs) -> np.ndarray:
    TA, TB, in_maps = _pack_inputs(inputs)
    key = (TA, TB)
    if key not in _PROG_CACHE:
        _PROG_CACHE[key] = _build_program(TA, TB)
    nc = _PROG_CACHE[key]
    res = run_bass_kernel_spmd(nc, in_maps, core_ids=list(range(NCORES)))
    return np.ascontiguousarray(res.results[0]["outT"].T).astype(F32)


# revision 17
# speedup vs baseline: 2.0119x; 1.0183x over previous
"""DimeNet-style GNN message passing on 8 Trainium2 NeuronCores.

Strategy
--------
Only rows dst<N of the [E,H] triplet-aggregation buffer are ever read
(agg_e[dst] with dst in [0,N)), so triplets with j_idx >= N are dead:
~40k of 640k triplets survive.

Sharding: core c owns node range [2048c, 2048(c+1)).  Triplets are
bucketed by j_idx//128 (node chunk), edges by dst//128.  All segment
sums become chunk-local one-hot matmuls accumulated in PSUM.  The only
collectives are an AllGather of per-node projections after layers 0/1
and a tiny AllReduce of pooled per-graph sums.

The gathered node table holds projections, not h: row n = [h[n]@W2a(l),
h[n]@W1h(l)] (bf16, 256B rows).  Stage A (triplets) is then
relu(hw1[k_idx] + rcproj) where rcproj = [rbf|cbf]@W1rc + b1 is
host-precomputed per layer, and stage B (edges) is
relu(hproj[src] + aggproj[dst]) - both pure elementwise on gathered
rows; no per-tile MLP matmuls.  Gathers use non-transpose dma_gather
(~9ns/idx of GpSimd descriptor generation - the kernel bottleneck).
"""
import sys

if '/opt/trn_rl_repo' not in sys.path:
    sys.path.insert(0, '/opt/trn_rl_repo')

import numpy as np
import ml_dtypes

import concourse.bacc as bacc
import concourse.bass as bass
import concourse.mybir as mybir
import concourse.tile as tile
from concourse.bass_utils import run_bass_kernel_spmd

BF16 = ml_dtypes.bfloat16
F32 = np.float32

N, E, T, B = 16000, 256000, 640000, 128
FIN, H, OUT, L = 64, 64, 32, 3
NCORES = 8
NCH = 16                 # node chunks per core (128 nodes each)
CN = NCH * 128           # 2048 nodes per core
NPAD = NCORES * CN       # 16384 padded node table rows
NCHG = NCORES * NCH      # 128 global chunks

AF = mybir.ActivationFunctionType
ALU = mybir.AluOpType
DT = mybir.dt


def _wrap_idx(ids: np.ndarray) -> np.ndarray:
    """dma_gather index layout: idx i -> [i%16, i//16], replicated to 128 partitions."""
    s = ids.shape[0]
    w = ids.reshape(s // 16, 16).T.astype(np.int16)
    return np.ascontiguousarray(np.tile(w, (8, 1)))


def _cpp() -> int:
    return max(1, NCH // 4)   # chunks per AllGather piece


def _rowmap(n: np.ndarray) -> np.ndarray:
    """Node id -> table row under the piece-major AllGather layout:
    piece p = chunk//cpp; row = p*(8*cpp*128) + core*(cpp*128)
    + (chunk%cpp)*128 + slot."""
    cpp = _cpp()
    c = n // CN
    r = n % CN
    ch = r // 128
    return ((ch // cpp) * (NCORES * cpp * 128) + c * (cpp * 128)
            + (ch % cpp) * 128 + (r % 128))


def _bucket(gchunk: np.ndarray, cap: int, nvals: int):
    """Slot position for each element: gchunk*cap + rank-within-chunk."""
    order = np.argsort(gchunk, kind='stable')
    sg = gchunk[order]
    starts = np.searchsorted(sg, np.arange(NCHG))
    rank = np.arange(len(sg)) - starts[sg]
    assert rank.max() < cap if len(rank) else True
    pos = sg * cap + rank
    out_pos = np.empty(nvals, np.int64)
    out_pos[order] = pos
    return out_pos


def _build_program(TA: int, TB: int):
    CAPA, CAPB = TA * 128, TB * 128
    nc = bacc.Bacc("TRN2", target_bir_lowering=False, debug=False,
                   num_devices=NCORES)

    # ---------------- DRAM I/O ----------------
    def din(name, shape, dt):
        return nc.dram_tensor(name, list(shape), dt, kind="ExternalInput")

    xpad = din("xpad", [NPAD, 128], DT.bfloat16)       # [x@W2a0 | x@W1h0]
    xT = din("xT", [64, CN], DT.bfloat16)
    a_idx = din("a_idx", [128, NCH * CAPA // 16], DT.int16)
    a_cmp = din("a_cmp", [128, NCH * TA], DT.bfloat16)
    a_rcp = din("a_rcp", [L, NCH * CAPA, 64], DT.bfloat16)
    b_idx = din("b_idx", [128, NCH * CAPB // 16], DT.int16)
    b_cmp = din("b_cmp", [128, NCH * TB], DT.bfloat16)
    b_cmprow = din("b_cmprow", [1, NCH * CAPB], DT.bfloat16)
    p_cmp = din("p_cmp", [128, NCH], DT.bfloat16)
    iota_r = din("iota_r", [128, 128], DT.bfloat16)
    iota_c = din("iota_c", [128, 1], DT.float32)
    ones_b = din("ones_b", [1, 128], DT.bfloat16)
    w1h = din("w1h", [L, 64, 64], DT.bfloat16)
    w2a = din("w2a", [L, 64, 64], DT.bfloat16)
    w2b = din("w2b", [L, 64, 64], DT.bfloat16)
    b2r = din("b2r", [L, 1, 64], DT.bfloat16)
    wn1a = din("wn1a", [L, 64, 64], DT.bfloat16)
    wn1b = din("wn1b", [L, 64, 64], DT.bfloat16)
    wn2 = din("wn2", [L, 64, 64], DT.bfloat16)
    bn1c = din("bn1c", [L, 64, 1], DT.float32)
    bn2c = din("bn2c", [L, 64, 1], DT.float32)
    bn2r = din("bn2r", [L, 1, 64], DT.bfloat16)
    wo1 = din("wo1", [64, 64], DT.float32)
    bo1c = din("bo1c", [64, 1], DT.float32)
    wo2 = din("wo2", [64, 32], DT.float32)
    bo2c = din("bo2c", [32, 1], DT.float32)
    invr = din("invr", [1, 128], DT.float32)
    outT = nc.dram_tensor("outT", [32, 128], DT.float32, kind="ExternalOutput")

    with tile.TileContext(nc) as tc:
        with (
            tc.tile_pool(name="res", bufs=1) as res,           # resident SBUF
            tc.tile_pool(name="gat", bufs=3) as gat,           # gather tiles
            tc.tile_pool(name="oh", bufs=4) as oh,             # one-hot tiles
            tc.tile_pool(name="sm", bufs=4) as sm,             # small evacuations
            tc.tile_pool(name="pw", bufs=3, space="PSUM") as pw,    # work psum
            tc.tile_pool(name="bc", bufs=1, space="PSUM") as bc,    # bcast psum
            tc.tile_pool(name="pa", bufs=3, space="PSUM") as pa,    # accumulators
            tc.tile_pool(name="pp", bufs=1, space="PSUM") as pp,    # pooled accum
            tc.tile_pool(name="dram", bufs=1, space="DRAM") as dram,
        ):
            # ---------- load resident data ----------
            def load(src, shape, dt, name):
                t = res.tile(list(shape), dt, name=name)
                nc.sync.dma_start(t[:], src)
                return t

            a_idx_s = load(a_idx.ap(), [128, NCH * CAPA // 16], DT.int16, "a_idx_s")
            a_cmp_s = load(a_cmp.ap(), [128, NCH * TA], DT.bfloat16, "a_cmp_s")
            b_idx_s = load(b_idx.ap(), [128, NCH * CAPB // 16], DT.int16, "b_idx_s")
            b_cmp_s = load(b_cmp.ap(), [128, NCH * TB], DT.bfloat16, "b_cmp_s")
            b_cmprow_s = load(b_cmprow.ap(), [1, NCH * CAPB], DT.bfloat16,
                              "b_cmprow_s")
            p_cmp_s = load(p_cmp.ap(), [128, NCH], DT.bfloat16, "p_cmp_s")
            iota_r_s = load(iota_r.ap(), [128, 128], DT.bfloat16, "iota_r_s")
            iota_c_s = load(iota_c.ap(), [128, 1], DT.float32, "iota_c_s")
            ones_b_s = load(ones_b.ap(), [1, 128], DT.bfloat16, "ones_b_s")

            def load_l(src, parts, width, dt, name):
                t = res.tile([parts, L * width], dt, name=name)
                for l in range(L):
                    nc.sync.dma_start(t[:, l * width:(l + 1) * width],
                                      src.ap()[l])
                return t

            w1h_s = load_l(w1h, 64, 64, DT.bfloat16, "w1h_s")
            w2a_s = load_l(w2a, 64, 64, DT.bfloat16, "w2a_s")
            w2b_s = load_l(w2b, 64, 64, DT.bfloat16, "w2b_s")
            b2r_s = load_l(b2r, 1, 64, DT.bfloat16, "b2r_s")
            wn1a_s = load_l(wn1a, 64, 64, DT.bfloat16, "wn1a_s")
            wn1b_s = load_l(wn1b, 64, 64, DT.bfloat16, "wn1b_s")
            wn2_s = load_l(wn2, 64, 64, DT.bfloat16, "wn2_s")
            bn1c_s = load_l(bn1c, 64, 1, DT.float32, "bn1c_s")
            bn2c_s = load_l(bn2c, 64, 1, DT.float32, "bn2c_s")
            bn2r_s = load_l(bn2r, 1, 64, DT.bfloat16, "bn2r_s")
            wo1_s = load(wo1.ap(), [64, 64], DT.float32, "wo1_s")
            bo1c_s = load(bo1c.ap(), [64, 1], DT.float32, "bo1c_s")
            wo2_s = load(wo2.ap(), [64, 32], DT.float32, "wo2_s")
            bo2c_s = load(bo2c.ap(), [32, 1], DT.float32, "bo2c_s")
            invr_s = load(invr.ap(), [1, 128], DT.float32, "invr_s")
            hT = load(xT.ap(), [64, CN], DT.bfloat16, "hT")

            # ---------- internal DRAM ----------
            cpp = _cpp()
            bounce = [[dram.tile([cpp * 128, 128], DT.bfloat16,
                                 name=f"bounce{l}_{p}")
                       for p in range(NCH // cpp)]
                      for l in range(L - 1)]
            tables = [dram.tile([NPAD, 128], DT.bfloat16, name=f"table{l}")
                      for l in range(1, L)]
            pb_in = dram.tile([64, 128], DT.float32, name="pb_in")
            pb_out = dram.tile([64, 128], DT.float32, name="pb_out")

            pooled_ps = pp.tile([64, 128], DT.float32, name="pooled_ps")

            def iota_mid(gw):
                ap = iota_r_s[:]
                return bass.AP(ap.tensor, ap.offset,
                               [ap.ap[0], [0, gw], ap.ap[1]])

            def gather(pool_name, table_ap, idx_s, base, cap):
                g = gat.tile([128, cap // 128, 128], DT.bfloat16, name=pool_name)
                for s in range(0, cap, 512):
                    w = min(512, cap - s)
                    nc.gpsimd.dma_gather(
                        g[:, s // 128:(s + w) // 128, :], table_ap,
                        idx_s[:, (base + s) // 16:(base + s + w) // 16],
                        w, w, 128, transpose=False)
                return g

            for l in range(L):
                table_ap = xpad.ap() if l == 0 else tables[l - 1][:, :]
                lw = slice(l * 64, (l + 1) * 64)
                lwn = slice((l + 1) * 64, (l + 2) * 64)   # next layer weights
                for ch in range(NCH):
                    # ===== stage A: em = relu(hw1[k] + rcproj); scatter to agg =====
                    agA = gather("agA", table_ap, a_idx_s, ch * CAPA, CAPA)
                    rcp = gat.tile([128, TA, 64], DT.bfloat16, name="rcp")
                    nc.sync.dma_start(
                        rcp[:],
                        a_rcp.ap()[l, ch * CAPA:(ch + 1) * CAPA, :].rearrange(
                            "(t p) f -> p t f", p=128))
                    agg_ps = pa.tile([64, 128], DT.float32, name="agg_ps",
                                     tag="acc")
                    # batched: one add + one relu + one one-hot build per chunk
                    em_sb = sm.tile([128, TA, 64], DT.bfloat16, name="em_sb")
                    nc.vector.tensor_tensor(em_sb[:], agA[:, :, 64:128],
                                            rcp[:], ALU.add)
                    nc.scalar.activation(em_sb[:], em_sb[:], AF.Relu)
                    sj = oh.tile([128, TA, 128], DT.bfloat16, name="sj")
                    nc.vector.tensor_tensor(
                        sj[:], iota_mid(TA),
                        a_cmp_s[:, ch * TA:(ch + 1) * TA].to_broadcast(
                            [128, TA, 128]), ALU.is_equal)
                    for t in range(TA):
                        nc.tensor.matmul(agg_ps[:], em_sb[:, t, :], sj[:, t, :],
                                         start=(t == 0), stop=(t == TA - 1))
                    aggT_sb = sm.tile([64, 128], DT.bfloat16, name="aggT_sb")
                    nc.scalar.activation(aggT_sb[:], agg_ps[:], AF.Copy)
                    apj_ps = pw.tile([128, 64], DT.float32, name="apj_ps", tag="w")
                    nc.tensor.matmul(apj_ps[:], ones_b_s[:], b2r_s[:, lw],
                                     start=True, stop=False)
                    nc.tensor.matmul(apj_ps[:], aggT_sb[:], w2b_s[:, lw],
                                     start=False, stop=True)
                    apj_sb = sm.tile([128, 64], DT.bfloat16, name="apj_sb")
                    nc.scalar.activation(apj_sb[:], apj_ps[:], AF.Copy)

                    # ===== stage B: fi = relu(hproj[src] + apj[dst]); scatter =====
                    agB = gather("agB", table_ap, b_idx_s, ch * CAPB, CAPB)
                    aggr_ps = pa.tile([64, 128], DT.float32, name="aggr_ps",
                                      tag="acc")
                    for g0 in range(0, TB, 4):
                        gw = min(4, TB - g0)
                        gt0 = ch * TB + g0
                        de4 = oh.tile([128, 4, 128], DT.bfloat16, name="de4")
                        nc.vector.tensor_tensor(
                            de4[:, :gw, :], iota_mid(gw),
                            b_cmp_s[:, gt0:gt0 + gw].to_broadcast(
                                [128, gw, 128]), ALU.is_equal)
                        bc_ps = bc.tile([128, 512], DT.float32, name="bc_ps")
                        nc.tensor.matmul(
                            bc_ps[:, :gw * 128], ones_b_s[:],
                            b_cmprow_s[:, gt0 * 128:(gt0 + gw) * 128],
                            start=True, stop=True)
                        dt4 = oh.tile([128, 4, 128], DT.bfloat16, name="dt4")
                        nc.vector.tensor_tensor(
                            dt4[:, :gw, :].rearrange("p t e -> p (t e)"),
                            bc_ps[:, :gw * 128],
                            iota_c_s[:].to_broadcast([128, gw * 128]),
                            ALU.is_equal)
                        fi_ps = pw.tile([128, 4, 64], DT.float32, name="fi_ps",
                                        tag="w")
                        for t in range(gw):
                            nc.tensor.matmul(fi_ps[:, t, :], dt4[:, t, :],
                                             apj_sb[:], start=True, stop=True)
                        fi_sb = sm.tile([128, 4, 64], DT.bfloat16, name="fi_sb")
                        nc.vector.tensor_tensor(fi_sb[:, :gw, :],
                                                fi_ps[:, :gw, :],
                                                agB[:, g0:g0 + gw, 0:64],
                                                ALU.add)
                        nc.scalar.activation(
                            fi_sb[:, :gw, :].rearrange("p t e -> p (t e)"),
                            fi_sb[:, :gw, :].rearrange("p t e -> p (t e)"),
                            AF.Relu)
                        for t in range(gw):
                            tt = g0 + t
                            nc.tensor.matmul(aggr_ps[:], fi_sb[:, t, :],
                                             de4[:, t, :],
                                             start=(tt == 0),
                                             stop=(tt == TB - 1))
                    aggr_sb = sm.tile([64, 128], DT.bfloat16, name="aggr_sb")
                    nc.scalar.activation(aggr_sb[:], aggr_ps[:], AF.Copy)

                    # ===== stage C: node update =====
                    chs = slice(ch * 128, (ch + 1) * 128)
                    z_ps = pw.tile([64, 128], DT.float32, name="z_ps", tag="w")
                    nc.tensor.matmul(z_ps[:], wn1a_s[:, lw], hT[:, chs],
                                     start=True, stop=False)
                    nc.tensor.matmul(z_ps[:], wn1b_s[:, lw], aggr_sb[:],
                                     start=False, stop=True)
                    z_sb = sm.tile([64, 128], DT.bfloat16, name="z_sb")
                    nc.scalar.activation(z_sb[:], z_ps[:], AF.Relu,
                                         bias=bn1c_s[:, l:l + 1])
                    hT_ps = pw.tile([64, 128], DT.float32, name="hT_ps", tag="w")
                    nc.tensor.matmul(hT_ps[:], wn2_s[:, lw], z_sb[:],
                                     start=True, stop=True)
                    nc.scalar.activation(hT[:, chs], hT_ps[:], AF.Identity,
                                         bias=bn2c_s[:, l:l + 1])
                    if l < L - 1:
                        # next-layer projections [h@W2a | h@W1h] -> bounce
                        pj_pad = sm.tile([128, 128], DT.bfloat16, name="pj_pad")
                        pj_ps = pw.tile([128, 64], DT.float32, name="pj_ps",
                                        tag="w")
                        nc.tensor.matmul(pj_ps[:], hT[:, chs], w2a_s[:, lwn],
                                         start=True, stop=True)
                        nc.scalar.activation(pj_pad[:, 0:64], pj_ps[:], AF.Copy)
                        pj2_ps = pw.tile([128, 64], DT.float32, name="pj2_ps",
                                         tag="w")
                        nc.tensor.matmul(pj2_ps[:], hT[:, chs], w1h_s[:, lwn],
                                         start=True, stop=True)
                        nc.scalar.activation(pj_pad[:, 64:128], pj2_ps[:],
                                             AF.Copy)
                        nc.sync.dma_start(
                            bounce[l][ch // cpp][(ch % cpp) * 128:
                                                 (ch % cpp + 1) * 128, :],
                            pj_pad[:])
                        if ch % cpp == cpp - 1:
                            p = ch // cpp
                            psz = NCORES * cpp * 128
                            nc.gpsimd.collective_compute(
                                "AllGather", ALU.bypass,
                                replica_groups=[list(range(NCORES))],
                                ins=[bounce[l][p].opt()],
                                outs=[tables[l][p * psz:(p + 1) * psz, :]])
                    else:
                        hn_ps = pw.tile([128, 64], DT.float32, name="hn_ps",
                                        tag="w")
                        nc.tensor.matmul(hn_ps[:], ones_b_s[:], bn2r_s[:, lw],
                                         start=True, stop=False)
                        nc.tensor.matmul(hn_ps[:], z_sb[:], wn2_s[:, lw],
                                         start=False, stop=True)
                        hn_sb = sm.tile([128, 64], DT.bfloat16, name="hn_sb")
                        nc.scalar.activation(hn_sb[:], hn_ps[:], AF.Copy)
                        bm = oh.tile([128, 128], DT.bfloat16, name="bm")
                        nc.vector.tensor_tensor(
                            bm[:], iota_r_s[:],
                            p_cmp_s[:, ch:ch + 1].to_broadcast([128, 128]),
                            ALU.is_equal)
                        nc.tensor.matmul(pooled_ps[:], hn_sb[:], bm[:],
                                         start=(ch == 0), stop=(ch == NCH - 1))

            # ================= pooling + head =================
            pooled_sb = res.tile([64, 128], DT.float32, name="pooled_sb")
            nc.scalar.activation(pooled_sb[:], pooled_ps[:], AF.Copy)
            nc.sync.dma_start(pb_in[:, :], pooled_sb[:])
            nc.gpsimd.collective_compute(
                "AllReduce", ALU.add, replica_groups=[list(range(NCORES))],
                ins=[pb_in.opt()], outs=[pb_out.opt()])
            pooled_all = res.tile([64, 128], DT.float32, name="pooled_all")
            nc.sync.dma_start(pooled_all[:], pb_out[:, :])
            ones_f = res.tile([1, 64], DT.float32, name="ones_f")
            nc.vector.memset(ones_f[:], 1.0)
            inv_ps = pw.tile([64, 128], DT.float32, name="inv_ps", tag="w")
            nc.tensor.matmul(inv_ps[:], ones_f[:], invr_s[:], start=True,
                             stop=True)
            pm_sb = res.tile([64, 128], DT.float32, name="pm_sb")
            nc.vector.tensor_tensor(pm_sb[:], pooled_all[:], inv_ps[:], ALU.mult)
            q_sb = res.tile([64, 128], DT.float32, name="q_sb")
            nc.scalar.activation(q_sb[:], pm_sb[:], AF.Relu)
            o1_ps = pw.tile([64, 128], DT.float32, name="o1_ps", tag="w")
            nc.tensor.matmul(o1_ps[:], wo1_s[:], q_sb[:], start=True, stop=True)
            o1_sb = res.tile([64, 128], DT.float32, name="o1_sb")
            nc.scalar.activation(o1_sb[:], o1_ps[:], AF.Relu, bias=bo1c_s[:])
            o2_ps = pw.tile([32, 128], DT.float32, name="o2_ps", tag="w")
            nc.tensor.matmul(o2_ps[:], wo2_s[:], o1_sb[:], start=True, stop=True)
            o2_sb = res.tile([32, 128], DT.float32, name="o2_sb")
            nc.scalar.activation(o2_sb[:], o2_ps[:], AF.Identity, bias=bo2c_s[:])
            nc.sync.dma_start(outT.ap(), o2_sb[:])
    nc.compile()
    return nc


def _pack_inputs(inputs):
    x = np.asarray(inputs["x"], F32)
    rbf = np.asarray(inputs["rbf"], F32)
    cbf = np.asarray(inputs["cbf"], F32)
    ei = np.asarray(inputs["edge_index"]).astype(np.int64)
    src, dst = ei[0], ei[1]
    k_idx = np.asarray(inputs["k_idx"]).astype(np.int64)
    j_idx = np.asarray(inputs["j_idx"]).astype(np.int64)
    batch = np.asarray(inputs["batch"]).astype(np.int64)
    W1 = np.asarray(inputs["W1"], F32); b1 = np.asarray(inputs["b1"], F32)
    W2 = np.asarray(inputs["W2"], F32); b2 = np.asarray(inputs["b2"], F32)
    Wn1 = np.asarray(inputs["Wn1"], F32); bn1 = np.asarray(inputs["bn1"], F32)
    Wn2 = np.asarray(inputs["Wn2"], F32); bn2 = np.asarray(inputs["bn2"], F32)
    Wo1 = np.asarray(inputs["Wo1"], F32); bo1 = np.asarray(inputs["bo1"], F32)
    Wo2 = np.asarray(inputs["Wo2"], F32); bo2 = np.asarray(inputs["bo2"], F32)

    # ---- triplet filtering + bucketing by j chunk ----
    keep = j_idx < N
    kk = k_idx[keep]; jk = j_idx[keep]
    rc = np.concatenate([rbf[jk], cbf[keep]], axis=1)   # [Tk, 12] f32

    gA = jk // 128
    cntA = np.bincount(gA, minlength=NCHG)
    TA = max(2, int(np.ceil(cntA.max() / 128)))
    CAPA = TA * 128
    posA = _bucket(gA, CAPA, len(jk))
    ak = np.zeros(NCHG * CAPA, np.int64); ak[posA] = _rowmap(kk)
    acmp = np.full(NCHG * CAPA, -1.0, F32); acmp[posA] = (jk % 128).astype(F32)
    # rcproj[l] = [rbf[j]|cbf] @ W1[l][64:76] + b1[l]  (host, layer-dependent)
    rcp = np.zeros((L, NCHG * CAPA, 64), BF16)
    for l in range(L):
        rcp[l][posA] = (rc @ W1[l, 64:76] + b1[l]).astype(BF16)

    # ---- edge bucketing by dst chunk ----
    gB = dst // 128
    cntB = np.bincount(gB, minlength=NCHG)
    TB = max(2, int(np.ceil(cntB.max() / 128)))
    CAPB = TB * 128
    posB = _bucket(gB, CAPB, E)
    bsrc = np.zeros(NCHG * CAPB, np.int64); bsrc[posB] = _rowmap(src)
    bcmp = np.full(NCHG * CAPB, -1.0, F32); bcmp[posB] = (dst % 128).astype(F32)

    # ---- shared (replicated) tensors ----
    # layer-0 table: [x@W2a0 | x@W1h0], quantization path matches the
    # device (bf16 operands, f32 accumulate, bf16 store)
    xpad = np.zeros((NPAD, 128), BF16)
    xq = x.astype(BF16).astype(F32)
    rows = _rowmap(np.arange(N))
    xpad[rows, 0:64] = (xq @ W2[0, :64].astype(BF16).astype(F32)).astype(BF16)
    xpad[rows, 64:128] = (xq @ W1[0, :64].astype(BF16).astype(F32)).astype(BF16)
    cnt = np.bincount(batch, minlength=128).astype(F32)[:128]
    invr = (1.0 / np.maximum(cnt, 1.0))[None, :].astype(F32)
    iota_r = np.ascontiguousarray(
        np.broadcast_to(np.arange(128, dtype=F32), (128, 128))).astype(BF16)
    iota_c = np.arange(128, dtype=F32)[:, None]
    ones_b = np.ones((1, 128), BF16)

    batch_pad = np.full(NPAD, -1.0, F32)
    batch_pad[:N] = batch.astype(F32)

    shared = dict(
        xpad=xpad, iota_r=iota_r, iota_c=iota_c, ones_b=ones_b,
        w1h=W1[:, :64].astype(BF16), w2a=W2[:, :64].astype(BF16),
        w2b=W2[:, 64:].astype(BF16), b2r=b2[:, None, :].astype(BF16),
        wn1a=Wn1[:, :64].astype(BF16), wn1b=Wn1[:, 64:].astype(BF16),
        wn2=Wn2.astype(BF16),
        bn1c=bn1[:, :, None].astype(F32), bn2c=bn2[:, :, None].astype(F32),
        bn2r=bn2[:, None, :].astype(BF16),
        wo1=Wo1, bo1c=bo1[:, None].astype(F32), wo2=Wo2,
        bo2c=bo2[:, None].astype(F32), invr=invr,
    )

    in_maps = []
    for c in range(NCORES):
        ts = slice(c * NCH * CAPA, (c + 1) * NCH * CAPA)   # triplet slots
        es = slice(c * NCH * CAPB, (c + 1) * NCH * CAPB)   # edge slots
        ns = slice(c * CN, (c + 1) * CN)                   # node slots
        xT_c = np.zeros((64, CN), BF16)
        nhi = min((c + 1) * CN, N)
        if nhi > c * CN:
            xT_c[:, :nhi - c * CN] = x[c * CN:nhi].T.astype(BF16)
        m = dict(shared)
        m.update(
            xT=xT_c,
            a_idx=_wrap_idx(ak[ts]),
            a_cmp=np.ascontiguousarray(
                acmp[ts].reshape(NCH * TA, 128).T).astype(BF16),
            a_rcp=np.ascontiguousarray(rcp[:, ts]),
            b_idx=_wrap_idx(bsrc[es]),
            b_cmp=np.ascontiguousarray(
                bcmp[es].reshape(NCH * TB, 128).T).astype(BF16),
            b_cmprow=bcmp[es][None, :].astype(BF16),
            p_cmp=np.ascontiguousarray(
                batch_pad[ns].reshape(NCH, 128).T).astype(BF16),
        )
        in_maps.append(m)
    return TA, TB, in_maps


_PROG_CACHE = {}


def kernel(**inputs) -> np.ndarray:
    TA, TB, in_maps = _pack_inputs(inputs)
    key = (TA, TB)
    if key not in _PROG_CACHE:
        _PROG_CACHE[key] = _build_program(TA, TB)
    nc = _PROG_CACHE[key]
    res = run_bass_kernel_spmd(nc, in_maps, core_ids=list(range(NCORES)))
    return np.ascontiguousarray(res.results[0]["outT"].T).astype(F32)


# revision 22
# speedup vs baseline: 2.0134x; 1.0008x over previous
"""DimeNet-style GNN message passing on 8 Trainium2 NeuronCores.

Strategy
--------
Only rows dst<N of the [E,H] triplet-aggregation buffer are ever read
(agg_e[dst] with dst in [0,N)), so triplets with j_idx >= N are dead:
~40k of 640k triplets survive.

Sharding: core c owns node range [2048c, 2048(c+1)).  Triplets are
bucketed by j_idx//128 (node chunk), edges by dst//128.  All segment
sums become chunk-local one-hot matmuls accumulated in PSUM.  The only
collectives are an AllGather of per-node projections after layers 0/1
and a tiny AllReduce of pooled per-graph sums.

The gathered node table holds projections, not h: row n = [h[n]@W2a(l),
h[n]@W1h(l)] (bf16, 256B rows).  Stage A (triplets) is then
relu(hw1[k_idx] + rcproj) where rcproj = [rbf|cbf]@W1rc + b1 is
host-precomputed per layer, and stage B (edges) is
relu(hproj[src] + aggproj[dst]) - both pure elementwise on gathered
rows; no per-tile MLP matmuls.  Gathers use non-transpose dma_gather
(~9ns/idx of GpSimd descriptor generation - the kernel bottleneck).
"""
import sys

if '/opt/trn_rl_repo' not in sys.path:
    sys.path.insert(0, '/opt/trn_rl_repo')

import numpy as np
import ml_dtypes

import concourse.bacc as bacc
import concourse.bass as bass
import concourse.mybir as mybir
import concourse.tile as tile
from concourse.bass_utils import run_bass_kernel_spmd

BF16 = ml_dtypes.bfloat16
F32 = np.float32
SKIP_PADS = False  # static gather counts (runtime-count variant blew the
                   # Pool register budget)

N, E, T, B = 16000, 256000, 640000, 128
FIN, H, OUT, L = 64, 64, 32, 3
NCORES = 8
NCH = 16                 # node chunks per core (128 nodes each)
CN = NCH * 128           # 2048 nodes per core
NPAD = NCORES * CN       # 16384 padded node table rows
NCHG = NCORES * NCH      # 128 global chunks

AF = mybir.ActivationFunctionType
ALU = mybir.AluOpType
DT = mybir.dt


def _wrap_idx(ids: np.ndarray) -> np.ndarray:
    """dma_gather index layout: idx i -> [i%16, i//16], replicated to 128 partitions."""
    s = ids.shape[0]
    w = ids.reshape(s // 16, 16).T.astype(np.int16)
    return np.ascontiguousarray(np.tile(w, (8, 1)))


def _pieces():
    """AllGather piece layout [(start_chunk, n_chunks), ...]; last piece is
    tiny so the layer-boundary serialization is short."""
    if NCH == 16:
        return [(0, 6), (6, 5), (11, 4), (15, 1)]
    return [(i, 1) for i in range(NCH)]


def _rowmap(n: np.ndarray) -> np.ndarray:
    """Node id -> table row under the piece-major AllGather layout."""
    pieces = _pieces()
    c = n // CN
    r = n % CN
    ch = r // 128
    s = r % 128
    row = np.zeros_like(n)
    base = 0
    for (p0, np_) in pieces:
        m = (ch >= p0) & (ch < p0 + np_)
        row[m] = (base + c[m] * (np_ * 128) + (ch[m] - p0) * 128 + s[m])
        base += NCORES * np_ * 128
    return row


def _bucket(gchunk: np.ndarray, cap: int, nvals: int):
    """Slot position for each element: gchunk*cap + rank-within-chunk."""
    order = np.argsort(gchunk, kind='stable')
    sg = gchunk[order]
    starts = np.searchsorted(sg, np.arange(NCHG))
    rank = np.arange(len(sg)) - starts[sg]
    assert rank.max() < cap if len(rank) else True
    pos = sg * cap + rank
    out_pos = np.empty(nvals, np.int64)
    out_pos[order] = pos
    return out_pos


def _build_program(TA: int, TB: int):
    CAPA, CAPB = TA * 128, TB * 128
    nc = bacc.Bacc("TRN2", target_bir_lowering=False, debug=False,
                   num_devices=NCORES)

    # ---------------- DRAM I/O ----------------
    def din(name, shape, dt):
        return nc.dram_tensor(name, list(shape), dt, kind="ExternalInput")

    xpad = din("xpad", [NPAD, 128], DT.bfloat16)       # [x@W2a0 | x@W1h0]
    xT = din("xT", [64, CN], DT.bfloat16)
    a_idx = din("a_idx", [128, NCH * CAPA // 16], DT.int16)
    a_cmp = din("a_cmp", [128, NCH * TA], DT.bfloat16)
    a_rcp = din("a_rcp", [L, NCH * CAPA, 64], DT.bfloat16)
    b_idx = din("b_idx", [128, NCH * CAPB // 16], DT.int16)
    b_cmp = din("b_cmp", [128, NCH * TB], DT.bfloat16)
    b_cmprow = din("b_cmprow", [1, NCH * CAPB], DT.bfloat16)
    p_cmp = din("p_cmp", [128, NCH], DT.bfloat16)
    iota_r = din("iota_r", [128, 128], DT.bfloat16)
    iota_c = din("iota_c", [128, 1], DT.float32)
    ones_b = din("ones_b", [1, 128], DT.bfloat16)
    w1h = din("w1h", [L, 64, 64], DT.bfloat16)
    w2a = din("w2a", [L, 64, 64], DT.bfloat16)
    w2b = din("w2b", [L, 64, 64], DT.bfloat16)
    b2r = din("b2r", [L, 1, 64], DT.bfloat16)
    wn1a = din("wn1a", [L, 64, 64], DT.bfloat16)
    wn1b = din("wn1b", [L, 64, 64], DT.bfloat16)
    wn2 = din("wn2", [L, 64, 64], DT.bfloat16)
    bn1c = din("bn1c", [L, 64, 1], DT.float32)
    bn2c = din("bn2c", [L, 64, 1], DT.float32)
    bn2r = din("bn2r", [L, 1, 64], DT.bfloat16)
    wo1 = din("wo1", [64, 64], DT.float32)
    bo1c = din("bo1c", [64, 1], DT.float32)
    wo2 = din("wo2", [64, 32], DT.float32)
    bo2c = din("bo2c", [32, 1], DT.float32)
    invr = din("invr", [1, 128], DT.float32)
    nca = (CAPA + 511) // 512
    ncb = (CAPB + 511) // 512
    gcnt = din("gcnt", [1, (nca + ncb) * NCH], DT.int32)
    outT = nc.dram_tensor("outT", [32, 128], DT.float32, kind="ExternalOutput")

    with tile.TileContext(nc) as tc:
        with (
            tc.tile_pool(name="res", bufs=1) as res,           # resident SBUF
            tc.tile_pool(name="gat", bufs=3) as gat,           # gather tiles
            tc.tile_pool(name="oh", bufs=4) as oh,             # one-hot tiles
            tc.tile_pool(name="sm", bufs=4) as sm,             # small evacuations
            tc.tile_pool(name="pw", bufs=3, space="PSUM") as pw,    # work psum
            tc.tile_pool(name="bc", bufs=1, space="PSUM") as bc,    # bcast psum
            tc.tile_pool(name="pa", bufs=3, space="PSUM") as pa,    # accumulators
            tc.tile_pool(name="pp", bufs=1, space="PSUM") as pp,    # pooled accum
            tc.tile_pool(name="dram", bufs=1, space="DRAM") as dram,
        ):
            # ---------- load resident data ----------
            def load(src, shape, dt, name):
                t = res.tile(list(shape), dt, name=name)
                nc.sync.dma_start(t[:], src)
                return t

            a_idx_s = load(a_idx.ap(), [128, NCH * CAPA // 16], DT.int16, "a_idx_s")
            a_cmp_s = load(a_cmp.ap(), [128, NCH * TA], DT.bfloat16, "a_cmp_s")
            b_idx_s = load(b_idx.ap(), [128, NCH * CAPB // 16], DT.int16, "b_idx_s")
            b_cmp_s = load(b_cmp.ap(), [128, NCH * TB], DT.bfloat16, "b_cmp_s")
            b_cmprow_s = load(b_cmprow.ap(), [1, NCH * CAPB], DT.bfloat16,
                              "b_cmprow_s")
            p_cmp_s = load(p_cmp.ap(), [128, NCH], DT.bfloat16, "p_cmp_s")
            iota_r_s = load(iota_r.ap(), [128, 128], DT.bfloat16, "iota_r_s")
            iota_c_s = load(iota_c.ap(), [128, 1], DT.float32, "iota_c_s")
            ones_b_s = load(ones_b.ap(), [1, 128], DT.bfloat16, "ones_b_s")

            def load_l(src, parts, width, dt, name):
                t = res.tile([parts, L * width], dt, name=name)
                for l in range(L):
                    nc.sync.dma_start(t[:, l * width:(l + 1) * width],
                                      src.ap()[l])
                return t

            w1h_s = load_l(w1h, 64, 64, DT.bfloat16, "w1h_s")
            w2a_s = load_l(w2a, 64, 64, DT.bfloat16, "w2a_s")
            w2b_s = load_l(w2b, 64, 64, DT.bfloat16, "w2b_s")
            b2r_s = load_l(b2r, 1, 64, DT.bfloat16, "b2r_s")
            wn1a_s = load_l(wn1a, 64, 64, DT.bfloat16, "wn1a_s")
            wn1b_s = load_l(wn1b, 64, 64, DT.bfloat16, "wn1b_s")
            wn2_s = load_l(wn2, 64, 64, DT.bfloat16, "wn2_s")
            bn1c_s = load_l(bn1c, 64, 1, DT.float32, "bn1c_s")
            bn2c_s = load_l(bn2c, 64, 1, DT.float32, "bn2c_s")
            bn2r_s = load_l(bn2r, 1, 64, DT.bfloat16, "bn2r_s")
            wo1_s = load(wo1.ap(), [64, 64], DT.float32, "wo1_s")
            bo1c_s = load(bo1c.ap(), [64, 1], DT.float32, "bo1c_s")
            wo2_s = load(wo2.ap(), [64, 32], DT.float32, "wo2_s")
            bo2c_s = load(bo2c.ap(), [32, 1], DT.float32, "bo2c_s")
            invr_s = load(invr.ap(), [1, 128], DT.float32, "invr_s")
            gcnt_s = load(gcnt.ap(), [1, (nca + ncb) * NCH], DT.int32, "gcnt_s")
            hT = load(xT.ap(), [64, CN], DT.bfloat16, "hT")

            # ---------- internal DRAM ----------
            pieces = _pieces()
            bounce = [[dram.tile([np_ * 128, 128], DT.bfloat16,
                                 name=f"bounce{l}_{p}")
                       for p, (p0, np_) in enumerate(pieces)]
                      for l in range(L - 1)]
            tables = [dram.tile([NPAD, 128], DT.bfloat16, name=f"table{l}")
                      for l in range(1, L)]
            pb_in = dram.tile([64, 128], DT.float32, name="pb_in")
            pb_out = dram.tile([64, 128], DT.float32, name="pb_out")

            pooled_ps = pp.tile([64, 128], DT.float32, name="pooled_ps")

            def iota_mid(gw):
                ap = iota_r_s[:]
                return bass.AP(ap.tensor, ap.offset,
                               [ap.ap[0], [0, gw], ap.ap[1]])

            def gather(pool_name, table_ap, idx_s, base, cap, cnt0,
                       first=False):
                g = gat.tile([128, cap // 128, 128], DT.bfloat16, name=pool_name)
                if first:
                    # first touch of this ring slot: pad slots the gather
                    # skips must hold finite values (0 x NaN poisons PSUM)
                    nc.vector.memset(g[:], 0.0)
                for ci, s in enumerate(range(0, cap, 512)):
                    w = min(512, cap - s)
                    nc.gpsimd.dma_gather(
                        g[:, s // 128:(s + w) // 128, :], table_ap,
                        idx_s[:, (base + s) // 16:(base + s + w) // 16],
                        w, w, 128, transpose=False)
                return g

            for l in range(L):
                table_ap = xpad.ap() if l == 0 else tables[l - 1][:, :]
                lw = slice(l * 64, (l + 1) * 64)
                lwn = slice((l + 1) * 64, (l + 2) * 64)   # next layer weights
                for ch in range(NCH):
                    # ===== stage A: em = relu(hw1[k] + rcproj); scatter to agg =====
                    agA = gather("agA", table_ap, a_idx_s, ch * CAPA, CAPA,
                                 ch * nca, first=(l == 0 and ch < 3))
                    rcp = gat.tile([128, TA, 64], DT.bfloat16, name="rcp")
                    nc.sync.dma_start(
                        rcp[:],
                        a_rcp.ap()[l, ch * CAPA:(ch + 1) * CAPA, :].rearrange(
                            "(t p) f -> p t f", p=128))
                    agg_ps = pa.tile([64, 128], DT.float32, name="agg_ps",
                                     tag="acc")
                    # batched: one add + one relu + one one-hot build per chunk
                    em_sb = sm.tile([128, TA, 64], DT.bfloat16, name="em_sb")
                    nc.vector.tensor_tensor(em_sb[:], agA[:, :, 64:128],
                                            rcp[:], ALU.add)
                    nc.scalar.activation(em_sb[:], em_sb[:], AF.Relu)
                    sj = oh.tile([128, TA, 128], DT.bfloat16, name="sj")
                    nc.vector.tensor_tensor(
                        sj[:], iota_mid(TA),
                        a_cmp_s[:, ch * TA:(ch + 1) * TA].to_broadcast(
                            [128, TA, 128]), ALU.is_equal)
                    for t in range(TA):
                        nc.tensor.matmul(agg_ps[:], em_sb[:, t, :], sj[:, t, :],
                                         start=(t == 0), stop=(t == TA - 1))
                    aggT_sb = sm.tile([64, 128], DT.bfloat16, name="aggT_sb")
                    nc.scalar.activation(aggT_sb[:], agg_ps[:], AF.Copy)
                    apj_ps = pw.tile([128, 64], DT.float32, name="apj_ps", tag="w")
                    nc.tensor.matmul(apj_ps[:], ones_b_s[:], b2r_s[:, lw],
                                     start=True, stop=False)
                    nc.tensor.matmul(apj_ps[:], aggT_sb[:], w2b_s[:, lw],
                                     start=False, stop=True)
                    apj_sb = sm.tile([128, 64], DT.bfloat16, name="apj_sb")
                    nc.scalar.activation(apj_sb[:], apj_ps[:], AF.Copy)

                    # ===== stage B: fi = relu(hproj[src] + apj[dst]); scatter =====
                    agB = gather("agB", table_ap, b_idx_s, ch * CAPB, CAPB,
                                 NCH * nca + ch * ncb,
                                 first=(l == 0 and ch < 3))
                    aggr_ps = pa.tile([64, 128], DT.float32, name="aggr_ps",
                                      tag="acc")
                    for g0 in range(0, TB, 4):
                        gw = min(4, TB - g0)
                        gt0 = ch * TB + g0
                        de4 = oh.tile([128, 4, 128], DT.bfloat16, name="de4")
                        nc.vector.tensor_tensor(
                            de4[:, :gw, :], iota_mid(gw),
                            b_cmp_s[:, gt0:gt0 + gw].to_broadcast(
                                [128, gw, 128]), ALU.is_equal)
                        bc_ps = bc.tile([128, 512], DT.float32, name="bc_ps")
                        nc.tensor.matmul(
                            bc_ps[:, :gw * 128], ones_b_s[:],
                            b_cmprow_s[:, gt0 * 128:(gt0 + gw) * 128],
                            start=True, stop=True)
                        dt4 = oh.tile([128, 4, 128], DT.bfloat16, name="dt4")
                        nc.vector.tensor_tensor(
                            dt4[:, :gw, :].rearrange("p t e -> p (t e)"),
                            bc_ps[:, :gw * 128],
                            iota_c_s[:].to_broadcast([128, gw * 128]),
                            ALU.is_equal)
                        fi_ps = pw.tile([128, 4, 64], DT.float32, name="fi_ps",
                                        tag="w")
                        for t in range(gw):
                            nc.tensor.matmul(fi_ps[:, t, :], dt4[:, t, :],
                                             apj_sb[:], start=True, stop=True)
                        fi_sb = sm.tile([128, 4, 64], DT.bfloat16, name="fi_sb")
                        nc.vector.tensor_tensor(fi_sb[:, :gw, :],
                                                fi_ps[:, :gw, :],
                                                agB[:, g0:g0 + gw, 0:64],
                                                ALU.add)
                        nc.scalar.activation(
                            fi_sb[:, :gw, :].rearrange("p t e -> p (t e)"),
                            fi_sb[:, :gw, :].rearrange("p t e -> p (t e)"),
                            AF.Relu)
                        for t in range(gw):
                            tt = g0 + t
                            nc.tensor.matmul(aggr_ps[:], fi_sb[:, t, :],
                                             de4[:, t, :],
                                             start=(tt == 0),
                                             stop=(tt == TB - 1))
                    aggr_sb = sm.tile([64, 128], DT.bfloat16, name="aggr_sb")
                    nc.scalar.activation(aggr_sb[:], aggr_ps[:], AF.Copy)

                    # ===== stage C: node update =====
                    chs = slice(ch * 128, (ch + 1) * 128)
                    z_ps = pw.tile([64, 128], DT.float32, name="z_ps", tag="w")
                    nc.tensor.matmul(z_ps[:], wn1a_s[:, lw], hT[:, chs],
                                     start=True, stop=False)
                    nc.tensor.matmul(z_ps[:], wn1b_s[:, lw], aggr_sb[:],
                                     start=False, stop=True)
                    z_sb = sm.tile([64, 128], DT.bfloat16, name="z_sb")
                    nc.scalar.activation(z_sb[:], z_ps[:], AF.Relu,
                                         bias=bn1c_s[:, l:l + 1])
                    hT_ps = pw.tile([64, 128], DT.float32, name="hT_ps", tag="w")
                    nc.tensor.matmul(hT_ps[:], wn2_s[:, lw], z_sb[:],
                                     start=True, stop=True)
                    nc.scalar.activation(hT[:, chs], hT_ps[:], AF.Identity,
                                         bias=bn2c_s[:, l:l + 1])
                    if l < L - 1:
                        # next-layer projections [h@W2a | h@W1h] -> bounce
                        pj_pad = sm.tile([128, 128], DT.bfloat16, name="pj_pad")
                        pj_ps = pw.tile([128, 64], DT.float32, name="pj_ps",
                                        tag="w")
                        nc.tensor.matmul(pj_ps[:], hT[:, chs], w2a_s[:, lwn],
                                         start=True, stop=True)
                        nc.scalar.activation(pj_pad[:, 0:64], pj_ps[:], AF.Copy)
                        pj2_ps = pw.tile([128, 64], DT.float32, name="pj2_ps",
                                         tag="w")
                        nc.tensor.matmul(pj2_ps[:], hT[:, chs], w1h_s[:, lwn],
                                         start=True, stop=True)
                        nc.scalar.activation(pj_pad[:, 64:128], pj2_ps[:],
                                             AF.Copy)
                        p = next(i for i, (p0, np_) in enumerate(pieces)
                                 if p0 <= ch < p0 + np_)
                        p0, np_ = pieces[p]
                        nc.sync.dma_start(
                            bounce[l][p][(ch - p0) * 128:
                                         (ch - p0 + 1) * 128, :],
                            pj_pad[:])
                        if ch == p0 + np_ - 1:
                            gbase = sum(q[1] for q in pieces[:p]) * NCORES * 128
                            psz = NCORES * np_ * 128
                            nc.gpsimd.collective_compute(
                                "AllGather", ALU.bypass,
                                replica_groups=[list(range(NCORES))],
                                ins=[bounce[l][p].opt()],
                                outs=[tables[l][gbase:gbase + psz, :]])
                    else:
                        hn_ps = pw.tile([128, 64], DT.float32, name="hn_ps",
                                        tag="w")
                        nc.tensor.matmul(hn_ps[:], ones_b_s[:], bn2r_s[:, lw],
                                         start=True, stop=False)
                        nc.tensor.matmul(hn_ps[:], z_sb[:], wn2_s[:, lw],
                                         start=False, stop=True)
                        hn_sb = sm.tile([128, 64], DT.bfloat16, name="hn_sb")
                        nc.scalar.activation(hn_sb[:], hn_ps[:], AF.Copy)
                        bm = oh.tile([128, 128], DT.bfloat16, name="bm")
                        nc.vector.tensor_tensor(
                            bm[:], iota_r_s[:],
                            p_cmp_s[:, ch:ch + 1].to_broadcast([128, 128]),
                            ALU.is_equal)
                        nc.tensor.matmul(pooled_ps[:], hn_sb[:], bm[:],
                                         start=(ch == 0), stop=(ch == NCH - 1))

            # ================= pooling + head =================
            pooled_sb = res.tile([64, 128], DT.float32, name="pooled_sb")
            nc.scalar.activation(pooled_sb[:], pooled_ps[:], AF.Copy)
            nc.sync.dma_start(pb_in[:, :], pooled_sb[:])
            nc.gpsimd.collective_compute(
                "AllReduce", ALU.add, replica_groups=[list(range(NCORES))],
                ins=[pb_in.opt()], outs=[pb_out.opt()])
            pooled_all = res.tile([64, 128], DT.float32, name="pooled_all")
            nc.sync.dma_start(pooled_all[:], pb_out[:, :])
            ones_f = res.tile([1, 64], DT.float32, name="ones_f")
            nc.vector.memset(ones_f[:], 1.0)
            inv_ps = pw.tile([64, 128], DT.float32, name="inv_ps", tag="w")
            nc.tensor.matmul(inv_ps[:], ones_f[:], invr_s[:], start=True,
                             stop=True)
            pm_sb = res.tile([64, 128], DT.float32, name="pm_sb")
            nc.vector.tensor_tensor(pm_sb[:], pooled_all[:], inv_ps[:], ALU.mult)
            q_sb = res.tile([64, 128], DT.float32, name="q_sb")
            nc.scalar.activation(q_sb[:], pm_sb[:], AF.Relu)
            o1_ps = pw.tile([64, 128], DT.float32, name="o1_ps", tag="w")
            nc.tensor.matmul(o1_ps[:], wo1_s[:], q_sb[:], start=True, stop=True)
            o1_sb = res.tile([64, 128], DT.float32, name="o1_sb")
            nc.scalar.activation(o1_sb[:], o1_ps[:], AF.Relu, bias=bo1c_s[:])
            o2_ps = pw.tile([32, 128], DT.float32, name="o2_ps", tag="w")
            nc.tensor.matmul(o2_ps[:], wo2_s[:], o1_sb[:], start=True, stop=True)
            o2_sb = res.tile([32, 128], DT.float32, name="o2_sb")
            nc.scalar.activation(o2_sb[:], o2_ps[:], AF.Identity, bias=bo2c_s[:])
            nc.sync.dma_start(outT.ap(), o2_sb[:])
    nc.compile()
    return nc


def _call_counts(real: np.ndarray, cap: int, idx_flat: np.ndarray):
    """Per-512-call valid counts for one core's chunks; ensures >=1 valid
    index per call (promoting one -1 pad to index 0 when a call is empty).
    idx_flat is modified in place (remaining -1 pads stay -1)."""
    ncalls = (cap + 511) // 512
    nch = real.shape[0]
    out = np.zeros(nch * ncalls, np.int32)
    for ch in range(nch):
        for ci in range(ncalls):
            s = ci * 512
            w = min(512, cap - s)
            if SKIP_PADS:
                nv = int(np.clip(real[ch] - s, 0, w))
                if nv == 0:
                    idx_flat[ch * cap + s] = 0
                    nv = 1
            else:
                sl = slice(ch * cap + s, ch * cap + s + w)
                idx_flat[sl] = np.maximum(idx_flat[sl], 0)
                nv = w
            out[ch * ncalls + ci] = nv
    return out


def _pack_inputs(inputs):
    x = np.asarray(inputs["x"], F32)
    rbf = np.asarray(inputs["rbf"], F32)
    cbf = np.asarray(inputs["cbf"], F32)
    ei = np.asarray(inputs["edge_index"]).astype(np.int64)
    src, dst = ei[0], ei[1]
    k_idx = np.asarray(inputs["k_idx"]).astype(np.int64)
    j_idx = np.asarray(inputs["j_idx"]).astype(np.int64)
    batch = np.asarray(inputs["batch"]).astype(np.int64)
    W1 = np.asarray(inputs["W1"], F32); b1 = np.asarray(inputs["b1"], F32)
    W2 = np.asarray(inputs["W2"], F32); b2 = np.asarray(inputs["b2"], F32)
    Wn1 = np.asarray(inputs["Wn1"], F32); bn1 = np.asarray(inputs["bn1"], F32)
    Wn2 = np.asarray(inputs["Wn2"], F32); bn2 = np.asarray(inputs["bn2"], F32)
    Wo1 = np.asarray(inputs["Wo1"], F32); bo1 = np.asarray(inputs["bo1"], F32)
    Wo2 = np.asarray(inputs["Wo2"], F32); bo2 = np.asarray(inputs["bo2"], F32)

    # ---- triplet filtering + bucketing by j chunk ----
    keep = j_idx < N
    kk = k_idx[keep]; jk = j_idx[keep]
    rc = np.concatenate([rbf[jk], cbf[keep]], axis=1)   # [Tk, 12] f32

    gA = jk // 128
    cntA = np.bincount(gA, minlength=NCHG)
    TA = max(2, int(np.ceil(cntA.max() / 128)))
    CAPA = TA * 128
    posA = _bucket(gA, CAPA, len(jk))
    ak = np.full(NCHG * CAPA, -1, np.int64); ak[posA] = _rowmap(kk)
    acmp = np.full(NCHG * CAPA, -1.0, F32); acmp[posA] = (jk % 128).astype(F32)
    # rcproj[l] = [rbf[j]|cbf] @ W1[l][64:76] + b1[l]  (host, layer-dependent)
    rcp = np.zeros((L, NCHG * CAPA, 64), BF16)
    for l in range(L):
        rcp[l][posA] = (rc @ W1[l, 64:76] + b1[l]).astype(BF16)

    # ---- edge bucketing by dst chunk ----
    gB = dst // 128
    cntB = np.bincount(gB, minlength=NCHG)
    TB = max(2, int(np.ceil(cntB.max() / 128)))
    CAPB = TB * 128
    posB = _bucket(gB, CAPB, E)
    bsrc = np.full(NCHG * CAPB, -1, np.int64); bsrc[posB] = _rowmap(src)
    bcmp = np.full(NCHG * CAPB, -1.0, F32); bcmp[posB] = (dst % 128).astype(F32)

    # ---- shared (replicated) tensors ----
    # layer-0 table: [x@W2a0 | x@W1h0], quantization path matches the
    # device (bf16 operands, f32 accumulate, bf16 store)
    xpad = np.zeros((NPAD, 128), BF16)
    xq = x.astype(BF16).astype(F32)
    rows = _rowmap(np.arange(N))
    xpad[rows, 0:64] = (xq @ W2[0, :64].astype(BF16).astype(F32)).astype(BF16)
    xpad[rows, 64:128] = (xq @ W1[0, :64].astype(BF16).astype(F32)).astype(BF16)
    cnt = np.bincount(batch, minlength=128).astype(F32)[:128]
    invr = (1.0 / np.maximum(cnt, 1.0))[None, :].astype(F32)
    iota_r = np.ascontiguousarray(
        np.broadcast_to(np.arange(128, dtype=F32), (128, 128))).astype(BF16)
    iota_c = np.arange(128, dtype=F32)[:, None]
    ones_b = np.ones((1, 128), BF16)

    batch_pad = np.full(NPAD, -1.0, F32)
    batch_pad[:N] = batch.astype(F32)

    shared = dict(
        xpad=xpad, iota_r=iota_r, iota_c=iota_c, ones_b=ones_b,
        w1h=W1[:, :64].astype(BF16), w2a=W2[:, :64].astype(BF16),
        w2b=W2[:, 64:].astype(BF16), b2r=b2[:, None, :].astype(BF16),
        wn1a=Wn1[:, :64].astype(BF16), wn1b=Wn1[:, 64:].astype(BF16),
        wn2=Wn2.astype(BF16),
        bn1c=bn1[:, :, None].astype(F32), bn2c=bn2[:, :, None].astype(F32),
        bn2r=bn2[:, None, :].astype(BF16),
        wo1=Wo1, bo1c=bo1[:, None].astype(F32), wo2=Wo2,
        bo2c=bo2[:, None].astype(F32), invr=invr,
    )

    in_maps = []
    for c in range(NCORES):
        ts = slice(c * NCH * CAPA, (c + 1) * NCH * CAPA)   # triplet slots
        es = slice(c * NCH * CAPB, (c + 1) * NCH * CAPB)   # edge slots
        ns = slice(c * CN, (c + 1) * CN)                   # node slots
        xT_c = np.zeros((64, CN), BF16)
        nhi = min((c + 1) * CN, N)
        if nhi > c * CN:
            xT_c[:, :nhi - c * CN] = x[c * CN:nhi].T.astype(BF16)
        akc = ak[ts].copy()
        bsc = bsrc[es].copy()
        gca = _call_counts(cntA[c * NCH:(c + 1) * NCH], CAPA, akc)
        gcb = _call_counts(cntB[c * NCH:(c + 1) * NCH], CAPB, bsc)
        m = dict(shared)
        m.update(
            gcnt=np.concatenate([gca, gcb])[None, :].astype(np.int32),
            xT=xT_c,
            a_idx=_wrap_idx(akc),
            a_cmp=np.ascontiguousarray(
                acmp[ts].reshape(NCH * TA, 128).T).astype(BF16),
            a_rcp=np.ascontiguousarray(rcp[:, ts]),
            b_idx=_wrap_idx(bsc),
            b_cmp=np.ascontiguousarray(
                bcmp[es].reshape(NCH * TB, 128).T).astype(BF16),
            b_cmprow=bcmp[es][None, :].astype(BF16),
            p_cmp=np.ascontiguousarray(
                batch_pad[ns].reshape(NCH, 128).T).astype(BF16),
        )
        in_maps.append(m)
    return TA, TB, in_maps


_PROG_CACHE = {}


def kernel(**inputs) -> np.ndarray:
    TA, TB, in_maps = _pack_inputs(inputs)
    key = (TA, TB)
    if key not in _PROG_CACHE:
        _PROG_CACHE[key] = _build_program(TA, TB)
    nc = _PROG_CACHE[key]
    res = run_bass_kernel_spmd(nc, in_maps, core_ids=list(range(NCORES)))
    return np.ascontiguousarray(res.results[0]["outT"].T).astype(F32)
